# revision 8
# baseline (speedup 1.0000x reference)
"""Trainium2 Bass kernel for nn_Decoder_Model (dense transformer decoder layer).

Sharding: data-parallel over batch (8 batches -> 8 cores). The three global
layernorms (normalized over ALL elements of the [B,S,D] tensor) need cross-core
scalar stats: each core computes local sum/sumsq, an 8-float AllReduce merges
them. AllReduce latency is hidden behind the next phase's matmuls using the
affine trick: norm(x)@W.T = (x@W.T)*rstd + per-channel-fix, so the big matmuls
run on raw x while stats are in flight and only a cheap fixup pass waits.

Perf notes vs the fp32r baseline:
- All matmul operands are bf16 (PSUM stays fp32): halves PE weight-load time,
  SBUF traffic and weight DMA; tolerance is 2e-2 so ~0.5% rounding is fine.
- Weights are transposed ONCE into resident SBUF (bf16), with psum->sbuf
  copy-outs grouped [128,512] and issued on gpsimd (scalar engine is reserved
  for softmax exp, its hard floor).
- Weight prep for later phases is pumped between attention heads so the PE
  never idles long enough to re-engage the HAM half-rate throttle.
- Cross-attn k/v projections are emitted before the q fix so AllReduce #1
  latency hides behind them.
"""
import sys

import numpy as np

sys.path.insert(0, "/opt/trn_rl_repo")

import concourse.bass as bass  # noqa: E402,F401
import concourse.mybir as mybir  # noqa: E402
import concourse.tile as tile  # noqa: E402
from concourse import bacc  # noqa: E402
from concourse import bass_utils  # noqa: E402
from concourse.masks import make_identity  # noqa: E402

F32 = mybir.dt.float32
F32R = mybir.dt.float32r
BF16 = mybir.dt.bfloat16
AF = mybir.ActivationFunctionType
OP = mybir.AluOpType

B, S, D, H, DK, FF = 8, 1024, 512, 8, 64, 2048
TT = S // 128   # 8 token tiles
DT = D // 128   # 4 feature tiles
FT = FF // 128  # 16 ffn tiles
TH = S // 512   # 2 matmul free-dim halves
N_CORES = 8
NTOT = float(B * S * D)
EPS = 1e-5

WNAMES = ["wq_m", "wk_m", "wv_m", "wo_m", "wq_c", "wk_c", "wv_c", "wo_c"]
BNAMES = ["bq_m", "bk_m", "bv_m", "bo_m", "bq_c", "bk_c", "bv_c", "bo_c"]

# self-attn causal chunking per k-tile kt over the q axis:
# (masked_chunk_start, masked_chunk_width, [(clean_start, clean_width), ...])
CAUSAL_CHUNKS = {
    0: (0, 256, [(256, 512), (768, 256)]),
    1: (128, 256, [(384, 384), (768, 256)]),
    2: (256, 256, [(512, 512)]),
    3: (384, 256, [(640, 384)]),
    4: (512, 256, [(768, 256)]),
    5: (640, 384, []),
    6: (768, 256, []),
    7: (896, 128, []),
}


def build_nc():
    nc = bacc.Bacc("TRN2", target_bir_lowering=False, debug=False,
                   enable_asserts=False, num_devices=N_CORES)
    inp = {}
    inp["data_dec"] = nc.dram_tensor("data_dec", [S, D], F32,
                                     kind="ExternalInput").ap()
    inp["encoder_out"] = nc.dram_tensor("encoder_out", [S, D], F32,
                                        kind="ExternalInput").ap()
    for w in WNAMES:
        inp[w] = nc.dram_tensor(w, [D, D], F32, kind="ExternalInput").ap()
    for b in BNAMES:
        inp[b] = nc.dram_tensor(b, [D], F32, kind="ExternalInput").ap()
    inp["wf1"] = nc.dram_tensor("wf1", [FF, D], F32, kind="ExternalInput").ap()
    inp["bf1"] = nc.dram_tensor("bf1", [FF], F32, kind="ExternalInput").ap()
    inp["wf2"] = nc.dram_tensor("wf2", [D, FF], F32, kind="ExternalInput").ap()
    inp["bf2"] = nc.dram_tensor("bf2", [D], F32, kind="ExternalInput").ap()
    out_d = nc.dram_tensor("out", [S, D], F32, kind="ExternalOutput").ap()

    with tile.TileContext(nc) as tc:
        build_body(nc, tc, inp, out_d)
    nc.finalize()
    return nc


def build_body(nc, tc, inp, out_d):
    import contextlib
    ctx = contextlib.ExitStack()
    with ctx:
        sb = ctx.enter_context(tc.tile_pool(name="sb", bufs=1))
        stg = ctx.enter_context(tc.tile_pool(name="stg", bufs=4))
        cp = ctx.enter_context(tc.tile_pool(name="cp", bufs=2))
        dram = ctx.enter_context(tc.tile_pool(name="dram", bufs=1, space="DRAM"))
        ps_a = ctx.enter_context(tc.tile_pool(name="ps_a", bufs=2, space="PSUM"))
        ps_b = ctx.enter_context(tc.tile_pool(name="ps_b", bufs=3, space="PSUM"))
        ps_pv = ctx.enter_context(tc.tile_pool(name="ps_pv", bufs=3, space="PSUM"))

        def psA():
            return ps_a.tile([128, 512], F32, tag="A", name="pA")

        def psB():
            return ps_b.tile([128, 512], F32, tag="B", name="pB")

        def psT():
            return ps_b.tile([128, 512], F32, tag="B", name="pT")

        def wstage():
            return stg.tile([128, 512], F32, tag="wstage", name="wstg")

        ident = sb.tile([128, 128], F32, tag="ident")
        make_identity(nc, ident[:])
        onesf = sb.tile([128, 1], F32, tag="onesf")
        nc.vector.memset(onesf[:], 1.0)

        # binary causal mask (keep q >= k within the diagonal chunk)
        mask_f = wstage()
        nc.vector.memset(mask_f[:], 1.0)
        nc.gpsimd.affine_select(out=mask_f[:], in_=mask_f[:],
                                compare_op=OP.is_ge, fill=0.0, base=0,
                                channel_multiplier=-1, pattern=[[1, 512]])
        mask_b = sb.tile([128, 512], BF16, tag="mask_b")
        nc.vector.tensor_copy(mask_b[:], mask_f[:])

        # ---- biases (f32, used as per-partition scalar operands) ----
        bias = {}
        for b in BNAMES + ["bf2"]:
            t = sb.tile([128, DT], F32, tag=f"{b}_sb")
            nc.sync.dma_start(t[:], inp[b].rearrange("(t p) -> p t", p=128))
            bias[b] = t
        bf1_sb = sb.tile([128, FT], F32, tag="bf1_sb")
        nc.sync.dma_start(bf1_sb[:], inp["bf1"].rearrange("(t p) -> p t", p=128))
        bv_full = {}
        for b in ("bv_m", "bv_c"):
            row = wstage()
            nc.sync.dma_start(row[0:1, :], inp[b][None, :])
            full = sb.tile([128, D], F32, tag=f"{b}_full")
            nc.gpsimd.partition_broadcast(full[:], row[0:1, :])
            bv_full[b] = full

        # column sums for the norm affine fixes
        wsum_qc = sb.tile([128, DT], F32, tag="wsum_qc")
        wsum_f1 = sb.tile([128, FT], F32, tag="wsum_f1")

        # ---- resident transposed weights (bf16) ----
        wT = {w: sb.tile([128, DT, D], BF16, tag=f"T_{w}", name=f"T_{w}")
              for w in WNAMES}
        wf1T = sb.tile([128, DT, FF], BF16, tag="T_wf1")
        wf2T = sb.tile([128, FT, D], BF16, tag="T_wf2")

        # ---- activations ----
        x_T = sb.tile([128, DT, S], BF16, tag="g_x")
        enc_T = sb.tile([128, DT, S], BF16, tag="g_enc")
        q_T = sb.tile([128, DT, S], BF16, tag="g_q")
        k_T = sb.tile([128, DT, S], BF16, tag="g_k")
        v_tok = sb.tile([128, TT, H * 65], BF16, tag="g_v")
        attn_T = sb.tile([128, DT, S], BF16, tag="g_attn")
        r1_T = sb.tile([128, DT, S], BF16, tag="g_r1")
        r2_T = sb.tile([128, DT, S], BF16, tag="g_r2")
        h_T = sb.tile([128, FT, 512], BF16, tag="g_h")
        r3_T = sb.tile([128, DT, S], F32, tag="g_r3")
        r3_tok = sb.tile([128, TT, D], F32, tag="g_r3tok")
        scr = sb.tile([128, 512], F32, tag="scr")

        # ---- transpose helpers ----
        def transpose_group4(dst_view, stage, wsum_col=None):
            """stage [128(rows),512(=4x128 cols)] -> 4 transposed blocks into
            one psum bank, one grouped gpsimd copy-out to dst_view
            ([128, 4, 128] view of a bf16 resident tile)."""
            pt = psT()
            for ki in range(4):
                nc.tensor.transpose(pt[:, ki * 128:(ki + 1) * 128],
                                    stage[:, ki * 128:(ki + 1) * 128], ident[:])
            nc.vector.tensor_copy(
                dst_view, pt[:].rearrange("p (k c) -> p k c", c=128))
            if wsum_col is not None:
                nc.vector.reduce_sum(wsum_col, stage[:],
                                     axis=mybir.AxisListType.X)

        def stage_dma(src_ap):
            stage = wstage()
            nc.sync.dma_start(stage[:], src_ap)
            return stage

        # background work queue: each closure emits one stage of weight prep
        bg = []

        def pump(n):
            for _ in range(min(n, len(bg))):
                bg.pop(0)()

        def prep_w_steps(wname, dst, wsum=None):
            """[512,512] weight -> dst [128, DT, 512] transposed bf16."""
            for ot in range(DT):
                def step(ot=ot):
                    stage = stage_dma(
                        inp[wname].rearrange("(t p) i -> p t i", p=128)[:, ot])
                    wcol = wsum[:, ot:ot + 1] if wsum is not None else None
                    transpose_group4(dst[:, :, ot * 128:(ot + 1) * 128],
                                     stage, wcol)
                bg.append(step)

        def prep_wf1_steps():
            for ot in range(FT):
                def step(ot=ot):
                    stage = stage_dma(
                        inp["wf1"].rearrange("(t p) i -> p t i", p=128)[:, ot])
                    transpose_group4(wf1T[:, :, ot * 128:(ot + 1) * 128],
                                     stage, wsum_f1[:, ot:ot + 1])
                bg.append(step)

        def prep_wf2_steps():
            for dd in range(DT):
                for piece in range(4):
                    def step(dd=dd, piece=piece):
                        stage = stage_dma(
                            inp["wf2"].rearrange("(t p) i -> p t i", p=128)
                            [:, dd, piece * 512:(piece + 1) * 512])
                        transpose_group4(
                            wf2T[:, piece * 4:piece * 4 + 4,
                                 dd * 128:(dd + 1) * 128], stage)
                    bg.append(step)

        def prep_act_steps(src_d, dst_T):
            """[S,D] activation -> dst_T [128, DT, S] bf16 feature-major."""
            for tt in range(TT):
                def step(tt=tt):
                    stage = stage_dma(
                        src_d.rearrange("(tt p) d -> p tt d", p=128)[:, tt])
                    transpose_group4(dst_T[:, :, tt * 128:(tt + 1) * 128],
                                     stage)
                bg.append(step)

        # ---- projection helpers ----
        def project_fm(w, src_T, out_tile, bias_tile=None,
                       fix_tile=None, scale_ap=None):
            """Feature-major projection: out[:, dd, :] = W^T-block @ src."""
            for dd in range(DT):
                for th in range(TH):
                    pt = psB()
                    for ki in range(DT):
                        nc.tensor.matmul(pt[:], wT[w][:, ki, dd * 128:(dd + 1) * 128],
                                         src_T[:, ki, th * 512:(th + 1) * 512],
                                         start=(ki == 0), stop=(ki == DT - 1))
                    dst = out_tile[:, dd, th * 512:(th + 1) * 512]
                    if fix_tile is not None:
                        nc.vector.tensor_scalar(dst, pt[:], scale_ap,
                                                fix_tile[:, dd:dd + 1],
                                                OP.mult, OP.add)
                    else:
                        nc.vector.tensor_scalar(dst, pt[:],
                                                bias_tile[:, dd:dd + 1], None,
                                                OP.add)

        def project_v(w, bname, src_T):
            """Token-major v with per-head ones column: v_tok [128,TT,H*65]."""
            ones_view = v_tok[:, :, :].rearrange(
                "p t (h c) -> p t h c", c=65)[:, :, :, 64]
            nc.vector.tensor_copy(
                ones_view, onesf[:, 0:1, None].to_broadcast([128, TT, H]))
            for tt in range(TT):
                pt = psB()
                for ki in range(DT):
                    nc.tensor.matmul(pt[:], src_T[:, ki, tt * 128:(tt + 1) * 128],
                                     wT[w][:, ki],
                                     start=(ki == 0), stop=(ki == DT - 1))
                dstv = v_tok[:, tt].rearrange("p (h c) -> p h c", c=65)[:, :, 0:64]
                nc.vector.tensor_tensor(
                    dstv, pt[:].rearrange("p (h c) -> p h c", c=64),
                    bv_full[bname][:].rearrange("p (h c) -> p h c", c=64),
                    OP.add)

        def attention(q_t, k_t, attn_t, causal, pump_n=0):
            for h in range(H):
                dt_, base = h // 2, (h % 2) * 64
                q_h = q_t[base:base + 64, dt_]
                k_h = k_t[base:base + 64, dt_]
                pv = {qh: ps_pv.tile([128, 512], F32, tag="PV", name="pPV")
                      for qh in range(TH)}
                for kt in range(TT):
                    pr = cp.tile([128, S], BF16, tag="probs", name="probs")
                    if causal:
                        m0, mw, clean = CAUSAL_CHUNKS[kt]
                        chunks = [(m0, mw, True)] + [(c0, cw, False)
                                                     for (c0, cw) in clean]
                    else:
                        chunks = [(0, 512, False), (512, 512, False)]
                    for (c0, cw, masked) in chunks:
                        st = psA()
                        nc.tensor.matmul(st[:, :cw],
                                         k_h[:, kt * 128:(kt + 1) * 128],
                                         q_h[:, c0:c0 + cw],
                                         start=True, stop=True)
                        nc.scalar.activation(pr[:, c0:c0 + cw], st[:, :cw],
                                             AF.Exp, scale=1.0 / 32.0)
                        if masked:
                            nc.vector.tensor_tensor(pr[:, c0:c0 + cw],
                                                    pr[:, c0:c0 + cw],
                                                    mask_b[:, 0:cw], OP.mult)
                    # PV contributions of this kt
                    v_h = v_tok[:, kt, h * 65:(h + 1) * 65]
                    for qh in range(TH):
                        if causal and qh == 0 and kt > 3:
                            continue
                        if causal:
                            off = max(0, (kt - qh * 4) * 128)
                            last = (kt == 3) if qh == 0 else (kt == 7)
                        else:
                            off, last = 0, (kt == 7)
                        nc.tensor.matmul(
                            pv[qh][:65, off:512], v_h,
                            pr[:, qh * 512 + off:(qh + 1) * 512],
                            start=(kt == 0), stop=last)
                # normalize by the rowsum living in row 64 of pv
                for qh in range(TH):
                    rec = cp.tile([1, 512], F32, tag="rsrec", name="rec", bufs=1)
                    nc.vector.reciprocal(rec[:], pv[qh][64:65, :])
                    rb = cp.tile([64, 512], F32, tag="rsbc", name="rb")
                    nc.gpsimd.partition_broadcast(rb[:], rec[:])
                    nc.vector.tensor_tensor(
                        attn_t[base:base + 64, dt_, qh * 512:(qh + 1) * 512],
                        pv[qh][0:64, :], rb[:], OP.mult)
                if pump_n:
                    pump(pump_n)

        def residual_out(w, src_T, bias_tile, res_T, out_T, stats_sb):
            """out_T = (W^T @ src_T) + bias + res_T ; accumulate sum/sumsq."""
            n_ki = src_T.shape[1]
            for dd in range(DT):
                for th in range(TH):
                    pt = psB()
                    for ki in range(n_ki):
                        nc.tensor.matmul(pt[:], wT[w][:, ki, dd * 128:(dd + 1) * 128],
                                         src_T[:, ki, th * 512:(th + 1) * 512],
                                         start=(ki == 0), stop=(ki == n_ki - 1))
                    dst = out_T[:, dd, th * 512:(th + 1) * 512]
                    c = dd * TH + th
                    nc.vector.scalar_tensor_tensor(
                        dst, pt[:], bias_tile[:, dd:dd + 1],
                        res_T[:, dd, th * 512:(th + 1) * 512],
                        OP.add, OP.add, accum_out=stats_sb[:, c:c + 1])
                    nc.vector.scalar_tensor_tensor(
                        scr[:], dst, 0.0, dst, OP.add, OP.mult,
                        accum_out=stats_sb[:, 8 + c:8 + c + 1])

        def stats_allreduce(stats_sb, name):
            pt = psA()
            nc.tensor.matmul(pt[:1, :16], onesf[:], stats_sb[:],
                             start=True, stop=True)
            red = sb.tile([1, 8], F32, tag=f"red_{name}")
            nc.vector.reduce_sum(red[:, 0:1], pt[0:1, 0:8],
                                 axis=mybir.AxisListType.X)
            nc.vector.reduce_sum(red[:, 1:2], pt[0:1, 8:16],
                                 axis=mybir.AxisListType.X)
            nc.vector.memset(red[:, 2:8], 0.0)
            ar_in = dram.tile([1, 8], F32, tag=f"ar_in_{name}")
            ar_out = dram.tile([1, 8], F32, tag=f"ar_out_{name}")
            nc.gpsimd.dma_start(ar_in[:], red[:])
            nc.gpsimd.collective_compute(
                "AllReduce", OP.add, replica_groups=[list(range(N_CORES))],
                ins=[ar_in.opt()], outs=[ar_out.opt()])
            g = sb.tile([1, 8], F32, tag=f"g_{name}")
            nc.sync.dma_start(g[:], ar_out[:])
            mu = sb.tile([1, 1], F32, tag=f"mu_{name}")
            nc.vector.tensor_scalar_mul(mu[:], g[:, 0:1], 1.0 / NTOT)
            ex2 = sb.tile([1, 1], F32, tag=f"ex2_{name}")
            nc.vector.tensor_scalar_mul(ex2[:], g[:, 1:2], 1.0 / NTOT)
            mu2 = sb.tile([1, 1], F32, tag=f"mu2_{name}")
            nc.vector.tensor_tensor(mu2[:], mu[:], mu[:], OP.mult)
            var = sb.tile([1, 1], F32, tag=f"var_{name}")
            nc.vector.tensor_tensor(var[:], ex2[:], mu2[:], OP.subtract)
            epst = sb.tile([1, 1], F32, tag=f"eps_{name}")
            nc.vector.memset(epst[:], EPS)
            std = sb.tile([1, 1], F32, tag=f"std_{name}")
            nc.scalar.activation(std[:], var[:], AF.Sqrt, bias=epst[:])
            rstd = sb.tile([1, 1], F32, tag=f"rstd_{name}")
            nc.vector.reciprocal(rstd[:], std[:])
            nmr = sb.tile([1, 1], F32, tag=f"nmr_{name}")
            nc.vector.tensor_tensor(nmr[:], mu[:], rstd[:], OP.mult)
            nc.vector.tensor_scalar_mul(nmr[:], nmr[:], -1.0)
            rstd_bc = sb.tile([128, 1], F32, tag=f"rstd_bc_{name}")
            nc.gpsimd.partition_broadcast(rstd_bc[:], rstd[:])
            nmr_bc = sb.tile([128, 1], F32, tag=f"nmr_bc_{name}")
            nc.gpsimd.partition_broadcast(nmr_bc[:], nmr[:])
            return rstd_bc, nmr_bc

        def materialize_norm(src_T, dst_T, rstd_bc, nmr_bc):
            for dd in range(DT):
                nc.vector.scalar_tensor_tensor(
                    dst_T[:, dd], src_T[:, dd], rstd_bc[:],
                    nmr_bc[:, :].to_broadcast([128, S]), OP.mult, OP.add)

        # ================= Phase 0: staged loads =================
        # x + self-attn weights emitted inline (they gate phase 1);
        # later weights go on the background queue, pumped between heads.
        prep_act_steps(inp["data_dec"], x_T)
        prep_w_steps("wq_m", wT["wq_m"])
        prep_w_steps("wk_m", wT["wk_m"])
        prep_w_steps("wv_m", wT["wv_m"])
        pump(len(bg))  # emit now: x, wq, wk, wv
        prep_w_steps("wo_m", wT["wo_m"])
        pump(2)

        project_fm("wq_m", x_T, q_T, bias_tile=bias["bq_m"])
        project_fm("wk_m", x_T, k_T, bias_tile=bias["bk_m"])
        project_v("wv_m", "bv_m", x_T)
        pump(len(bg))  # rest of wo_m

        # queue cross-attn weights + enc for pumping inside self-attention
        prep_act_steps(inp["encoder_out"], enc_T)
        prep_w_steps("wk_c", wT["wk_c"])
        prep_w_steps("wv_c", wT["wv_c"])
        prep_w_steps("wq_c", wT["wq_c"], wsum=wsum_qc)
        prep_w_steps("wo_c", wT["wo_c"])

        # ================= Phase 1: self attention =================
        attention(q_T, k_T, attn_T, causal=True, pump_n=3)
        pump(len(bg))

        stats1 = sb.tile([128, 16], F32, tag="stats1")
        residual_out("wo_m", attn_T, bias["bo_m"], x_T, r1_T, stats1)
        rstd1, nmr1 = stats_allreduce(stats1, "n1")

        # ================= Phase 2: cross attention =================
        # k/v projections (enc-only) overlap the AllReduce; q's norm fix is
        # applied in the psum->sbuf step so only that vector op waits.
        project_fm("wk_c", enc_T, k_T, bias_tile=bias["bk_c"])
        project_v("wv_c", "bv_c", enc_T)
        qfix = sb.tile([128, DT], F32, tag="qfix")
        for dd in range(DT):
            nc.vector.scalar_tensor_tensor(
                qfix[:, dd:dd + 1], wsum_qc[:, dd:dd + 1], nmr1[:],
                bias["bq_c"][:, dd:dd + 1], OP.mult, OP.add)
        project_fm("wq_c", r1_T, q_T, fix_tile=qfix, scale_ap=rstd1)

        # queue FFN weights for pumping inside cross-attention
        prep_wf1_steps()
        prep_wf2_steps()

        attention(q_T, k_T, attn_T, causal=False, pump_n=4)
        pump(len(bg))

        nmm_T = sb.tile([128, DT, S], BF16, tag="g_x")     # reuses x_T space
        materialize_norm(r1_T, nmm_T, rstd1, nmr1)
        stats2 = sb.tile([128, 16], F32, tag="stats2")
        residual_out("wo_c", attn_T, bias["bo_c"], nmm_T, r2_T, stats2)
        rstd2, nmr2 = stats_allreduce(stats2, "n2")

        # ================= Phase 3: FFN =================
        ffix = sb.tile([128, FT], F32, tag="ffix")
        for ft in range(FT):
            nc.vector.scalar_tensor_tensor(
                ffix[:, ft:ft + 1], wsum_f1[:, ft:ft + 1], nmr2[:],
                bf1_sb[:, ft:ft + 1], OP.mult, OP.add)
        nmh_T = sb.tile([128, DT, S], BF16, tag="g_enc")   # reuses enc_T space
        materialize_norm(r2_T, nmh_T, rstd2, nmr2)

        stats3 = sb.tile([128, 16], F32, tag="stats3")
        for th in range(TH):
            for ft in range(FT):
                pt = psB()
                for ki in range(DT):
                    nc.tensor.matmul(pt[:], wf1T[:, ki, ft * 128:(ft + 1) * 128],
                                     r2_T[:, ki, th * 512:(th + 1) * 512],
                                     start=(ki == 0), stop=(ki == DT - 1))
                nc.scalar.activation(h_T[:, ft], pt[:], AF.Relu,
                                     bias=ffix[:, ft:ft + 1], scale=rstd2[:])
            for dd in range(DT):
                pt = psB()
                for ki in range(FT):
                    nc.tensor.matmul(pt[:], wf2T[:, ki, dd * 128:(dd + 1) * 128],
                                     h_T[:, ki],
                                     start=(ki == 0), stop=(ki == FT - 1))
                dst = r3_T[:, dd, th * 512:(th + 1) * 512]
                c = dd * TH + th
                nc.vector.scalar_tensor_tensor(
                    dst, pt[:], bias["bf2"][:, dd:dd + 1],
                    nmh_T[:, dd, th * 512:(th + 1) * 512], OP.add, OP.add,
                    accum_out=stats3[:, c:c + 1])
                nc.vector.scalar_tensor_tensor(
                    scr[:], dst, 0.0, dst, OP.add, OP.mult,
                    accum_out=stats3[:, 8 + c:8 + c + 1])

        # transpose r3 to token-major (overlaps AllReduce #3)
        for tt in range(TT):
            pt = psT()
            for dd in range(DT):
                nc.tensor.transpose(pt[:, dd * 128:(dd + 1) * 128],
                                    r3_T[:, dd, tt * 128:(tt + 1) * 128], ident[:])
            nc.vector.tensor_copy(
                r3_tok[:, tt].rearrange("p (k c) -> p k c", c=128),
                pt[:].rearrange("p (k c) -> p k c", c=128))

        rstd3, nmr3 = stats_allreduce(stats3, "n3")
        for tt in range(TT):
            nc.vector.scalar_tensor_tensor(
                r3_tok[:, tt], r3_tok[:, tt], rstd3[:],
                nmr3[:, :].to_broadcast([128, D]), OP.mult, OP.add)
            nc.sync.dma_start(
                out_d.rearrange("(tt p) d -> p tt d", p=128)[:, tt],
                r3_tok[:, tt])


_NC_CACHE = {}


def kernel(**inputs):
    if "nc" not in _NC_CACHE:
        _NC_CACHE["nc"] = build_nc()
    nc = _NC_CACHE["nc"]
    in_maps = []
    for b in range(N_CORES):
        m = {"data_dec": np.ascontiguousarray(
                 np.asarray(inputs["data_dec"], dtype=np.float32)[b]),
             "encoder_out": np.ascontiguousarray(
                 np.asarray(inputs["encoder_out"], dtype=np.float32)[b])}
        for k, v in inputs.items():
            if k not in ("data_dec", "encoder_out"):
                m[k] = np.ascontiguousarray(np.asarray(v, dtype=np.float32))
        in_maps.append(m)
    res = bass_utils.run_bass_kernel_spmd(nc, in_maps,
                                          core_ids=list(range(N_CORES)))
    return np.stack([res.results[b]["out"] for b in range(N_CORES)], axis=0)


# revision 11
# speedup vs baseline: 1.0170x; 1.0170x over previous
"""Trainium2 Bass kernel for nn_Decoder_Model (dense transformer decoder layer).

Sharding: data-parallel over batch (8 batches -> 8 cores). The three global
layernorms (normalized over ALL elements of the [B,S,D] tensor) need cross-core
scalar stats: each core computes local sum/sumsq, an 8-float AllReduce merges
them. AllReduce latency is hidden behind the next phase's matmuls using the
affine trick: norm(x)@W.T = (x@W.T)*rstd + per-channel-fix, so the big matmuls
run on raw x while stats are in flight and only a cheap fixup pass waits.

Perf notes vs the fp32r baseline:
- All matmul operands are bf16 (PSUM stays fp32): halves PE weight-load time,
  SBUF traffic and weight DMA; tolerance is 2e-2 so ~0.5% rounding is fine.
- Weights are transposed ONCE into resident SBUF (bf16), with psum->sbuf
  copy-outs grouped [128,512] and issued on gpsimd (scalar engine is reserved
  for softmax exp, its hard floor).
- Weight prep for later phases is pumped between attention heads so the PE
  never idles long enough to re-engage the HAM half-rate throttle.
- Cross-attn k/v projections are emitted before the q fix so AllReduce #1
  latency hides behind them.
"""
import sys

import numpy as np

sys.path.insert(0, "/opt/trn_rl_repo")

import concourse.bass as bass  # noqa: E402,F401
import concourse.mybir as mybir  # noqa: E402
import concourse.tile as tile  # noqa: E402
from concourse import bacc  # noqa: E402
from concourse import bass_utils  # noqa: E402
from concourse.masks import make_identity  # noqa: E402

F32 = mybir.dt.float32
F32R = mybir.dt.float32r
BF16 = mybir.dt.bfloat16
AF = mybir.ActivationFunctionType
OP = mybir.AluOpType

B, S, D, H, DK, FF = 8, 1024, 512, 8, 64, 2048
TT = S // 128   # 8 token tiles
DT = D // 128   # 4 feature tiles
FT = FF // 128  # 16 ffn tiles
TH = S // 512   # 2 matmul free-dim halves
N_CORES = 8
NTOT = float(B * S * D)
EPS = 1e-5

WNAMES = ["wq_m", "wk_m", "wv_m", "wo_m", "wq_c", "wk_c", "wv_c", "wo_c"]
BNAMES = ["bq_m", "bk_m", "bv_m", "bo_m", "bq_c", "bk_c", "bv_c", "bo_c"]

# self-attn causal chunking per k-tile kt over the q axis:
# (masked_chunk_start, masked_chunk_width, [(clean_start, clean_width), ...])
CAUSAL_CHUNKS = {
    0: (0, 256, [(256, 512), (768, 256)]),
    1: (128, 256, [(384, 384), (768, 256)]),
    2: (256, 256, [(512, 512)]),
    3: (384, 256, [(640, 384)]),
    4: (512, 256, [(768, 256)]),
    5: (640, 384, []),
    6: (768, 256, []),
    7: (896, 128, []),
}


def build_nc():
    nc = bacc.Bacc("TRN2", target_bir_lowering=False, debug=False,
                   enable_asserts=False, num_devices=N_CORES)
    inp = {}
    inp["data_dec"] = nc.dram_tensor("data_dec", [S, D], F32,
                                     kind="ExternalInput").ap()
    inp["encoder_out"] = nc.dram_tensor("encoder_out", [S, D], F32,
                                        kind="ExternalInput").ap()
    for w in WNAMES:
        inp[w] = nc.dram_tensor(w, [D, D], F32, kind="ExternalInput").ap()
    for b in BNAMES:
        inp[b] = nc.dram_tensor(b, [D], F32, kind="ExternalInput").ap()
    inp["wf1"] = nc.dram_tensor("wf1", [FF, D], F32, kind="ExternalInput").ap()
    inp["bf1"] = nc.dram_tensor("bf1", [FF], F32, kind="ExternalInput").ap()
    inp["wf2"] = nc.dram_tensor("wf2", [D, FF], F32, kind="ExternalInput").ap()
    inp["bf2"] = nc.dram_tensor("bf2", [D], F32, kind="ExternalInput").ap()
    out_d = nc.dram_tensor("out", [S, D], F32, kind="ExternalOutput").ap()

    with tile.TileContext(nc) as tc:
        build_body(nc, tc, inp, out_d)
    nc.finalize()
    return nc


def build_body(nc, tc, inp, out_d):
    import contextlib
    ctx = contextlib.ExitStack()
    with ctx:
        sb = ctx.enter_context(tc.tile_pool(name="sb", bufs=1))
        stg = ctx.enter_context(tc.tile_pool(name="stg", bufs=4))
        cp = ctx.enter_context(tc.tile_pool(name="cp", bufs=2))
        dram = ctx.enter_context(tc.tile_pool(name="dram", bufs=1, space="DRAM"))
        ps_a = ctx.enter_context(tc.tile_pool(name="ps_a", bufs=2, space="PSUM"))
        ps_b = ctx.enter_context(tc.tile_pool(name="ps_b", bufs=3, space="PSUM"))
        ps_pv = ctx.enter_context(tc.tile_pool(name="ps_pv", bufs=3, space="PSUM"))

        def psA():
            return ps_a.tile([128, 512], F32, tag="A", name="pA")

        def psB():
            return ps_b.tile([128, 512], F32, tag="B", name="pB")

        def psT():
            return ps_b.tile([128, 512], F32, tag="B", name="pT")

        def wstage():
            return stg.tile([128, 512], F32, tag="wstage", name="wstg")

        ident = sb.tile([128, 128], F32, tag="ident")
        make_identity(nc, ident[:])
        ident_b = sb.tile([128, 128], BF16, tag="ident_b")
        nc.vector.tensor_copy(ident_b[:], ident[:])
        onesf = sb.tile([128, 1], F32, tag="onesf")
        nc.vector.memset(onesf[:], 1.0)

        # binary causal mask (keep q >= k within the diagonal chunk)
        mask_f = wstage()
        nc.vector.memset(mask_f[:], 1.0)
        nc.gpsimd.affine_select(out=mask_f[:], in_=mask_f[:],
                                compare_op=OP.is_ge, fill=0.0, base=0,
                                channel_multiplier=-1, pattern=[[1, 512]])
        mask_b = sb.tile([128, 512], BF16, tag="mask_b")
        nc.vector.tensor_copy(mask_b[:], mask_f[:])

        # ---- biases (f32, used as per-partition scalar operands) ----
        bias = {}
        for b in BNAMES + ["bf2"]:
            t = sb.tile([128, DT], F32, tag=f"{b}_sb")
            nc.sync.dma_start(t[:], inp[b].rearrange("(t p) -> p t", p=128))
            bias[b] = t
        bf1_sb = sb.tile([128, FT], F32, tag="bf1_sb")
        nc.sync.dma_start(bf1_sb[:], inp["bf1"].rearrange("(t p) -> p t", p=128))
        bv_full = {}
        for b in ("bv_m", "bv_c"):
            row = wstage()
            nc.sync.dma_start(row[0:1, :], inp[b][None, :])
            full = sb.tile([128, D], F32, tag=f"{b}_full")
            nc.gpsimd.partition_broadcast(full[:], row[0:1, :])
            bv_full[b] = full

        # column sums for the norm affine fixes
        wsum_qc = sb.tile([128, DT], F32, tag="wsum_qc")
        wsum_f1 = sb.tile([128, FT], F32, tag="wsum_f1")

        # ---- resident transposed weights (bf16) ----
        wT = {w: sb.tile([128, DT, D], BF16, tag=f"T_{w}", name=f"T_{w}")
              for w in WNAMES}
        wf1T = sb.tile([128, DT, FF], BF16, tag="T_wf1")
        wf2T = sb.tile([128, FT, D], BF16, tag="T_wf2")

        # ---- activations ----
        x_T = sb.tile([128, DT, S], BF16, tag="g_x")
        enc_T = sb.tile([128, DT, S], BF16, tag="g_enc")
        q_T = sb.tile([128, DT, S], BF16, tag="g_q")
        k_T = sb.tile([128, DT, S], BF16, tag="g_k")
        v_tok = sb.tile([128, TT, H * 65], BF16, tag="g_v")
        attn_T = sb.tile([128, DT, S], BF16, tag="g_attn")
        r1_T = sb.tile([128, DT, S], BF16, tag="g_r1")
        r2_T = sb.tile([128, DT, S], BF16, tag="g_r2")
        h_T = sb.tile([128, FT, S], BF16, tag="g_h")
        r3_T = sb.tile([128, DT, S], BF16, tag="g_r3")
        r3_tok = sb.tile([128, TT, D], BF16, tag="g_r3tok")
        scr = sb.tile([128, 512], F32, tag="scr")

        # ---- transpose helpers ----
        def transpose_group4(dst_view, stage, wsum_col=None, engine="vector"):
            """stage [128(rows),512(=4x128 cols)] -> 4 transposed blocks into
            one psum bank, one grouped copy-out to dst_view
            ([128, 4, 128] view of a bf16 resident tile)."""
            pt = psT()
            for ki in range(4):
                nc.tensor.transpose(pt[:, ki * 128:(ki + 1) * 128],
                                    stage[:, ki * 128:(ki + 1) * 128], ident[:])
            pv_ = pt[:].rearrange("p (k c) -> p k c", c=128)
            if engine == "scalar":
                nc.scalar.copy(dst_view, pv_)
            else:
                nc.vector.tensor_copy(dst_view, pv_)
            if wsum_col is not None:
                nc.vector.reduce_sum(wsum_col, stage[:],
                                     axis=mybir.AxisListType.X)

        def stage_dma(src_ap):
            stage = wstage()
            nc.sync.dma_start(stage[:], src_ap)
            return stage

        # background work queue: each closure emits one stage of weight prep
        bg = []

        def pump(n):
            for _ in range(min(n, len(bg))):
                bg.pop(0)()

        def prep_w_steps(wname, dst, wsum=None):
            """[512,512] weight -> dst [128, DT, 512] transposed bf16."""
            for ot in range(DT):
                def step(ot=ot):
                    stage = stage_dma(
                        inp[wname].rearrange("(t p) i -> p t i", p=128)[:, ot])
                    wcol = wsum[:, ot:ot + 1] if wsum is not None else None
                    transpose_group4(dst[:, :, ot * 128:(ot + 1) * 128],
                                     stage, wcol)
                bg.append(step)

        def prep_wf1_steps():
            for ot in range(FT):
                def step(ot=ot):
                    stage = stage_dma(
                        inp["wf1"].rearrange("(t p) i -> p t i", p=128)[:, ot])
                    transpose_group4(wf1T[:, :, ot * 128:(ot + 1) * 128],
                                     stage, wsum_f1[:, ot:ot + 1])
                bg.append(step)

        def prep_wf2_steps():
            for dd in range(DT):
                for piece in range(4):
                    def step(dd=dd, piece=piece):
                        stage = stage_dma(
                            inp["wf2"].rearrange("(t p) i -> p t i", p=128)
                            [:, dd, piece * 512:(piece + 1) * 512])
                        transpose_group4(
                            wf2T[:, piece * 4:piece * 4 + 4,
                                 dd * 128:(dd + 1) * 128], stage)
                    bg.append(step)

        def prep_act_steps(src_d, dst_T, engine="vector"):
            """[S,D] activation -> dst_T [128, DT, S] bf16 feature-major."""
            for tt in range(TT):
                def step(tt=tt):
                    stage = stage_dma(
                        src_d.rearrange("(tt p) d -> p tt d", p=128)[:, tt])
                    transpose_group4(dst_T[:, :, tt * 128:(tt + 1) * 128],
                                     stage, engine=engine)
                bg.append(step)

        # ---- projection helpers ----
        def project_fm(w, src_T, out_tile, bias_tile=None, out_engine="vector"):
            """Feature-major projection: out[:, dd, :] = W^T-block @ src.
            bias_tile=None leaves the output raw (bias/norm fix applied later
            in-place, so an AllReduce wait never backs up the PSUM banks)."""
            for dd in range(DT):
                for th in range(TH):
                    pt = psB()
                    for ki in range(DT):
                        nc.tensor.matmul(pt[:], wT[w][:, ki, dd * 128:(dd + 1) * 128],
                                         src_T[:, ki, th * 512:(th + 1) * 512],
                                         start=(ki == 0), stop=(ki == DT - 1))
                    dst = out_tile[:, dd, th * 512:(th + 1) * 512]
                    if out_engine == "scalar":
                        if bias_tile is None:
                            nc.scalar.copy(dst, pt[:])
                        else:
                            nc.scalar.activation(dst, pt[:], AF.Identity,
                                                 bias=bias_tile[:, dd:dd + 1])
                    else:
                        if bias_tile is None:
                            nc.vector.tensor_copy(dst, pt[:])
                        else:
                            nc.vector.tensor_scalar(dst, pt[:],
                                                    bias_tile[:, dd:dd + 1],
                                                    None, OP.add)

        def project_v(w, bname, src_T):
            """Token-major v with per-head ones column: v_tok [128,TT,H*65]."""
            ones_view = v_tok[:, :, :].rearrange(
                "p t (h c) -> p t h c", c=65)[:, :, :, 64]
            nc.vector.tensor_copy(
                ones_view, onesf[:, 0:1, None].to_broadcast([128, TT, H]))
            for tt in range(TT):
                pt = psB()
                for ki in range(DT):
                    nc.tensor.matmul(pt[:], src_T[:, ki, tt * 128:(tt + 1) * 128],
                                     wT[w][:, ki],
                                     start=(ki == 0), stop=(ki == DT - 1))
                dstv = v_tok[:, tt].rearrange("p (h c) -> p h c", c=65)[:, :, 0:64]
                nc.vector.tensor_tensor(
                    dstv, pt[:].rearrange("p (h c) -> p h c", c=64),
                    bv_full[bname][:].rearrange("p (h c) -> p h c", c=64),
                    OP.add)

        def attention(q_t, k_t, attn_t, causal, pump_n=0):
            for h in range(H):
                dt_, base = h // 2, (h % 2) * 64
                q_h = q_t[base:base + 64, dt_]
                k_h = k_t[base:base + 64, dt_]
                pv = {qh: ps_pv.tile([128, 512], F32, tag="PV", name="pPV")
                      for qh in range(TH)}
                for kt in range(TT):
                    pr = cp.tile([128, S], BF16, tag="probs", name="probs")
                    if causal:
                        m0, mw, clean = CAUSAL_CHUNKS[kt]
                        chunks = [(m0, mw, True)] + [(c0, cw, False)
                                                     for (c0, cw) in clean]
                    else:
                        chunks = [(0, 512, False), (512, 512, False)]
                    for (c0, cw, masked) in chunks:
                        st = psA()
                        nc.tensor.matmul(st[:, :cw],
                                         k_h[:, kt * 128:(kt + 1) * 128],
                                         q_h[:, c0:c0 + cw],
                                         start=True, stop=True)
                        nc.scalar.activation(pr[:, c0:c0 + cw], st[:, :cw],
                                             AF.Exp, scale=1.0 / 32.0)
                        if masked:
                            nc.vector.tensor_tensor(pr[:, c0:c0 + cw],
                                                    pr[:, c0:c0 + cw],
                                                    mask_b[:, 0:cw], OP.mult)
                    # PV contributions of this kt
                    v_h = v_tok[:, kt, h * 65:(h + 1) * 65]
                    for qh in range(TH):
                        if causal and qh == 0 and kt > 3:
                            continue
                        if causal:
                            off = max(0, (kt - qh * 4) * 128)
                            last = (kt == 3) if qh == 0 else (kt == 7)
                        else:
                            off, last = 0, (kt == 7)
                        nc.tensor.matmul(
                            pv[qh][:65, off:512], v_h,
                            pr[:, qh * 512 + off:(qh + 1) * 512],
                            start=(kt == 0), stop=last)
                # normalize by the rowsum living in row 64 of pv
                for qh in range(TH):
                    rec = cp.tile([1, 512], F32, tag="rsrec", name="rec", bufs=1)
                    nc.vector.reciprocal(rec[:], pv[qh][64:65, :])
                    rb = cp.tile([64, 512], F32, tag="rsbc", name="rb")
                    nc.gpsimd.partition_broadcast(rb[:], rec[:])
                    nc.vector.tensor_tensor(
                        attn_t[base:base + 64, dt_, qh * 512:(qh + 1) * 512],
                        pv[qh][0:64, :], rb[:], OP.mult)
                if pump_n:
                    pump(pump_n)

        def residual_out(w, src_T, bias_tile, res_T, out_T, stats_sb):
            """out_T = (W^T @ src_T) + bias + res_T ; accumulate sum/sumsq."""
            n_ki = src_T.shape[1]
            for dd in range(DT):
                for th in range(TH):
                    pt = psB()
                    for ki in range(n_ki):
                        nc.tensor.matmul(pt[:], wT[w][:, ki, dd * 128:(dd + 1) * 128],
                                         src_T[:, ki, th * 512:(th + 1) * 512],
                                         start=(ki == 0), stop=(ki == n_ki - 1))
                    dst = out_T[:, dd, th * 512:(th + 1) * 512]
                    c = dd * TH + th
                    nc.vector.scalar_tensor_tensor(
                        dst, pt[:], bias_tile[:, dd:dd + 1],
                        res_T[:, dd, th * 512:(th + 1) * 512],
                        OP.add, OP.add, accum_out=stats_sb[:, c:c + 1])
                    nc.vector.scalar_tensor_tensor(
                        scr[:], dst, 0.0, dst, OP.add, OP.mult,
                        accum_out=stats_sb[:, 8 + c:8 + c + 1])

        def stats_ar_kick(stats_sb, name):
            pt = psA()
            nc.tensor.matmul(pt[:1, :16], onesf[:], stats_sb[:],
                             start=True, stop=True)
            red = sb.tile([1, 8], F32, tag=f"red_{name}")
            nc.vector.reduce_sum(red[:, 0:1], pt[0:1, 0:8],
                                 axis=mybir.AxisListType.X)
            nc.vector.reduce_sum(red[:, 1:2], pt[0:1, 8:16],
                                 axis=mybir.AxisListType.X)
            nc.vector.memset(red[:, 2:8], 0.0)
            ar_in = dram.tile([1, 8], F32, tag=f"ar_in_{name}")
            ar_out = dram.tile([1, 8], F32, tag=f"ar_out_{name}")
            nc.gpsimd.dma_start(ar_in[:], red[:])
            nc.gpsimd.collective_compute(
                "AllReduce", OP.add, replica_groups=[list(range(N_CORES))],
                ins=[ar_in.opt()], outs=[ar_out.opt()])
            g = sb.tile([1, 8], F32, tag=f"g_{name}")
            nc.sync.dma_start(g[:], ar_out[:])
            return g

        def stats_ar_finish(g, name):
            mu = sb.tile([1, 1], F32, tag=f"mu_{name}")
            nc.vector.tensor_scalar_mul(mu[:], g[:, 0:1], 1.0 / NTOT)
            ex2 = sb.tile([1, 1], F32, tag=f"ex2_{name}")
            nc.vector.tensor_scalar_mul(ex2[:], g[:, 1:2], 1.0 / NTOT)
            mu2 = sb.tile([1, 1], F32, tag=f"mu2_{name}")
            nc.vector.tensor_tensor(mu2[:], mu[:], mu[:], OP.mult)
            var = sb.tile([1, 1], F32, tag=f"var_{name}")
            nc.vector.tensor_tensor(var[:], ex2[:], mu2[:], OP.subtract)
            epst = sb.tile([1, 1], F32, tag=f"eps_{name}")
            nc.vector.memset(epst[:], EPS)
            std = sb.tile([1, 1], F32, tag=f"std_{name}")
            nc.scalar.activation(std[:], var[:], AF.Sqrt, bias=epst[:])
            rstd = sb.tile([1, 1], F32, tag=f"rstd_{name}")
            nc.vector.reciprocal(rstd[:], std[:])
            nmr = sb.tile([1, 1], F32, tag=f"nmr_{name}")
            nc.vector.tensor_tensor(nmr[:], mu[:], rstd[:], OP.mult)
            nc.vector.tensor_scalar_mul(nmr[:], nmr[:], -1.0)
            rstd_bc = sb.tile([128, 1], F32, tag=f"rstd_bc_{name}")
            nc.gpsimd.partition_broadcast(rstd_bc[:], rstd[:])
            nmr_bc = sb.tile([128, 1], F32, tag=f"nmr_bc_{name}")
            nc.gpsimd.partition_broadcast(nmr_bc[:], nmr[:])
            return rstd_bc, nmr_bc

        def materialize_norm(src_T, dst_T, rstd_bc, nmr_bc):
            for dd in range(DT):
                nc.vector.scalar_tensor_tensor(
                    dst_T[:, dd], src_T[:, dd], rstd_bc[:],
                    nmr_bc[:, :].to_broadcast([128, S]), OP.mult, OP.add)

        # ================= Phase 0: staged loads =================
        # x + self-attn weights emitted inline (they gate phase 1);
        # later weights go on the background queue, pumped between heads.
        prep_act_steps(inp["data_dec"], x_T, engine="scalar")
        prep_w_steps("wq_m", wT["wq_m"])
        prep_w_steps("wk_m", wT["wk_m"])
        prep_w_steps("wv_m", wT["wv_m"])
        pump(len(bg))  # emit now: x, wq, wk, wv
        prep_w_steps("wo_m", wT["wo_m"])
        pump(2)

        project_fm("wq_m", x_T, q_T, bias_tile=bias["bq_m"])
        project_fm("wk_m", x_T, k_T, bias_tile=bias["bk_m"],
                   out_engine="scalar")
        project_v("wv_m", "bv_m", x_T)
        pump(len(bg))  # rest of wo_m

        # queue cross-attn weights + enc for pumping inside self-attention
        prep_act_steps(inp["encoder_out"], enc_T)
        prep_w_steps("wk_c", wT["wk_c"])
        prep_w_steps("wv_c", wT["wv_c"])
        prep_w_steps("wq_c", wT["wq_c"], wsum=wsum_qc)
        prep_w_steps("wo_c", wT["wo_c"])

        # ================= Phase 1: self attention =================
        attention(q_T, k_T, attn_T, causal=True, pump_n=3)
        pump(len(bg))

        stats1 = sb.tile([128, 16], F32, tag="stats1")
        residual_out("wo_m", attn_T, bias["bo_m"], x_T, r1_T, stats1)
        g1 = stats_ar_kick(stats1, "n1")

        # ================= Phase 2: cross attention =================
        # k/v/q-raw projections (no AR dependency) overlap the AllReduce;
        # q's norm fix lands in-place afterwards so PSUM never backs up.
        project_fm("wk_c", enc_T, k_T, bias_tile=bias["bk_c"])
        project_v("wv_c", "bv_c", enc_T)
        project_fm("wq_c", r1_T, q_T, bias_tile=None, out_engine="scalar")
        rstd1, nmr1 = stats_ar_finish(g1, "n1")
        qfix = sb.tile([128, DT], F32, tag="qfix")
        for dd in range(DT):
            nc.vector.scalar_tensor_tensor(
                qfix[:, dd:dd + 1], wsum_qc[:, dd:dd + 1], nmr1[:],
                bias["bq_c"][:, dd:dd + 1], OP.mult, OP.add)
        for dd in range(DT):
            nc.vector.tensor_scalar(q_T[:, dd], q_T[:, dd], rstd1[:],
                                    qfix[:, dd:dd + 1], OP.mult, OP.add)

        # queue FFN weights for pumping inside cross-attention
        prep_wf1_steps()
        prep_wf2_steps()

        attention(q_T, k_T, attn_T, causal=False, pump_n=4)
        pump(len(bg))

        nmm_T = sb.tile([128, DT, S], BF16, tag="g_x")     # reuses x_T space
        materialize_norm(r1_T, nmm_T, rstd1, nmr1)
        stats2 = sb.tile([128, 16], F32, tag="stats2")
        residual_out("wo_c", attn_T, bias["bo_c"], nmm_T, r2_T, stats2)
        g2 = stats_ar_kick(stats2, "n2")

        # ================= Phase 3: FFN =================
        # ff1 raw matmuls for BOTH halves run during the AllReduce; the
        # relu+scale pass (AR-dependent) is applied in-place on h_T.
        for th in range(TH):
            for ft in range(FT):
                pt = psB()
                for ki in range(DT):
                    nc.tensor.matmul(pt[:], wf1T[:, ki, ft * 128:(ft + 1) * 128],
                                     r2_T[:, ki, th * 512:(th + 1) * 512],
                                     start=(ki == 0), stop=(ki == DT - 1))
                nc.vector.tensor_copy(h_T[:, ft, th * 512:(th + 1) * 512],
                                      pt[:])
        rstd2, nmr2 = stats_ar_finish(g2, "n2")
        ffix = sb.tile([128, FT], F32, tag="ffix")
        for ft in range(FT):
            nc.vector.scalar_tensor_tensor(
                ffix[:, ft:ft + 1], wsum_f1[:, ft:ft + 1], nmr2[:],
                bf1_sb[:, ft:ft + 1], OP.mult, OP.add)
        for ft in range(FT):
            nc.scalar.activation(h_T[:, ft], h_T[:, ft], AF.Relu,
                                 bias=ffix[:, ft:ft + 1], scale=rstd2[:])
        nmh_T = sb.tile([128, DT, S], BF16, tag="g_enc")   # reuses enc_T space
        materialize_norm(r2_T, nmh_T, rstd2, nmr2)

        stats3 = sb.tile([128, 16], F32, tag="stats3")
        for th in range(TH):
            for dd in range(DT):
                pt = psB()
                for ki in range(FT):
                    nc.tensor.matmul(pt[:], wf2T[:, ki, dd * 128:(dd + 1) * 128],
                                     h_T[:, ki, th * 512:(th + 1) * 512],
                                     start=(ki == 0), stop=(ki == FT - 1))
                dst = r3_T[:, dd, th * 512:(th + 1) * 512]
                c = dd * TH + th
                nc.vector.scalar_tensor_tensor(
                    dst, pt[:], bias["bf2"][:, dd:dd + 1],
                    nmh_T[:, dd, th * 512:(th + 1) * 512], OP.add, OP.add,
                    accum_out=stats3[:, c:c + 1])
                nc.vector.scalar_tensor_tensor(
                    scr[:], dst, 0.0, dst, OP.add, OP.mult,
                    accum_out=stats3[:, 8 + c:8 + c + 1])
        g3 = stats_ar_kick(stats3, "n3")

        # transpose r3 to token-major (overlaps AllReduce #3)
        for tt in range(TT):
            pt = ps_b.tile([128, 512], BF16, tag="B", name="pT3")
            for dd in range(DT):
                nc.tensor.transpose(pt[:, dd * 128:(dd + 1) * 128],
                                    r3_T[:, dd, tt * 128:(tt + 1) * 128], ident_b[:])
            nc.vector.tensor_copy(
                r3_tok[:, tt].rearrange("p (k c) -> p k c", c=128),
                pt[:].rearrange("p (k c) -> p k c", c=128))

        rstd3, nmr3 = stats_ar_finish(g3, "n3")
        for tt in range(TT):
            ost = wstage()
            nc.vector.scalar_tensor_tensor(
                ost[:], r3_tok[:, tt], rstd3[:],
                nmr3[:, :].to_broadcast([128, D]), OP.mult, OP.add)
            nc.sync.dma_start(
                out_d.rearrange("(tt p) d -> p tt d", p=128)[:, tt],
                ost[:])


_NC_CACHE = {}


def kernel(**inputs):
    if "nc" not in _NC_CACHE:
        _NC_CACHE["nc"] = build_nc()
    nc = _NC_CACHE["nc"]
    in_maps = []
    for b in range(N_CORES):
        m = {"data_dec": np.ascontiguousarray(
                 np.asarray(inputs["data_dec"], dtype=np.float32)[b]),
             "encoder_out": np.ascontiguousarray(
                 np.asarray(inputs["encoder_out"], dtype=np.float32)[b])}
        for k, v in inputs.items():
            if k not in ("data_dec", "encoder_out"):
                m[k] = np.ascontiguousarray(np.asarray(v, dtype=np.float32))
        in_maps.append(m)
    res = bass_utils.run_bass_kernel_spmd(nc, in_maps,
                                          core_ids=list(range(N_CORES)))
    return np.stack([res.results[b]["out"] for b in range(N_CORES)], axis=0)


# revision 12
# speedup vs baseline: 1.0392x; 1.0218x over previous
"""Trainium2 Bass kernel for nn_Decoder_Model (dense transformer decoder layer).

Sharding: data-parallel over batch (8 batches -> 8 cores). The three global
layernorms (normalized over ALL elements of the [B,S,D] tensor) need cross-core
scalar stats: each core computes local sum/sumsq, an 8-float AllReduce merges
them. AllReduce latency is hidden behind the next phase's matmuls using the
affine trick: norm(x)@W.T = (x@W.T)*rstd + per-channel-fix, so the big matmuls
run on raw x while stats are in flight and only a cheap fixup pass waits.

Perf notes vs the fp32r baseline:
- All matmul operands are bf16 (PSUM stays fp32): halves PE weight-load time,
  SBUF traffic and weight DMA; tolerance is 2e-2 so ~0.5% rounding is fine.
- Weights are transposed ONCE into resident SBUF (bf16), with psum->sbuf
  copy-outs grouped [128,512] and issued on gpsimd (scalar engine is reserved
  for softmax exp, its hard floor).
- Weight prep for later phases is pumped between attention heads so the PE
  never idles long enough to re-engage the HAM half-rate throttle.
- Cross-attn k/v projections are emitted before the q fix so AllReduce #1
  latency hides behind them.
"""
import sys

import numpy as np

sys.path.insert(0, "/opt/trn_rl_repo")

import concourse.bass as bass  # noqa: E402,F401
import concourse.mybir as mybir  # noqa: E402
import concourse.tile as tile  # noqa: E402
from concourse import bacc  # noqa: E402
from concourse import bass_utils  # noqa: E402
from concourse.masks import make_identity  # noqa: E402

F32 = mybir.dt.float32
F32R = mybir.dt.float32r
BF16 = mybir.dt.bfloat16
AF = mybir.ActivationFunctionType
OP = mybir.AluOpType

B, S, D, H, DK, FF = 8, 1024, 512, 8, 64, 2048
TT = S // 128   # 8 token tiles
DT = D // 128   # 4 feature tiles
FT = FF // 128  # 16 ffn tiles
TH = S // 512   # 2 matmul free-dim halves
N_CORES = 8
NTOT = float(B * S * D)
EPS = 1e-5

WNAMES = ["wq_m", "wk_m", "wv_m", "wo_m", "wq_c", "wk_c", "wv_c", "wo_c"]
BNAMES = ["bq_m", "bk_m", "bv_m", "bo_m", "bq_c", "bk_c", "bv_c", "bo_c"]

# self-attn causal chunking per k-tile kt over the q axis:
# (masked_chunk_start, masked_chunk_width, [(clean_start, clean_width), ...])
CAUSAL_CHUNKS = {
    0: (0, 256, [(256, 512), (768, 256)]),
    1: (128, 256, [(384, 384), (768, 256)]),
    2: (256, 256, [(512, 512)]),
    3: (384, 256, [(640, 384)]),
    4: (512, 256, [(768, 256)]),
    5: (640, 384, []),
    6: (768, 256, []),
    7: (896, 128, []),
}


def build_nc():
    nc = bacc.Bacc("TRN2", target_bir_lowering=False, debug=False,
                   enable_asserts=False, num_devices=N_CORES)
    inp = {}
    inp["data_dec"] = nc.dram_tensor("data_dec", [S, D], F32,
                                     kind="ExternalInput").ap()
    inp["encoder_out"] = nc.dram_tensor("encoder_out", [S, D], F32,
                                        kind="ExternalInput").ap()
    for w in WNAMES:
        inp[w] = nc.dram_tensor(w, [D, D], F32, kind="ExternalInput").ap()
    for b in BNAMES:
        inp[b] = nc.dram_tensor(b, [D], F32, kind="ExternalInput").ap()
    inp["wf1"] = nc.dram_tensor("wf1", [FF, D], F32, kind="ExternalInput").ap()
    inp["bf1"] = nc.dram_tensor("bf1", [FF], F32, kind="ExternalInput").ap()
    inp["wf2"] = nc.dram_tensor("wf2", [D, FF], F32, kind="ExternalInput").ap()
    inp["bf2"] = nc.dram_tensor("bf2", [D], F32, kind="ExternalInput").ap()
    out_d = nc.dram_tensor("out", [S, D], F32, kind="ExternalOutput").ap()

    with tile.TileContext(nc) as tc:
        build_body(nc, tc, inp, out_d)
    nc.finalize()
    return nc


def build_body(nc, tc, inp, out_d):
    import contextlib
    ctx = contextlib.ExitStack()
    with ctx:
        sb = ctx.enter_context(tc.tile_pool(name="sb", bufs=1))
        stg = ctx.enter_context(tc.tile_pool(name="stg", bufs=4))
        cp = ctx.enter_context(tc.tile_pool(name="cp", bufs=2))
        dram = ctx.enter_context(tc.tile_pool(name="dram", bufs=1, space="DRAM"))
        ps_a = ctx.enter_context(tc.tile_pool(name="ps_a", bufs=3, space="PSUM"))
        ps_b = ctx.enter_context(tc.tile_pool(name="ps_b", bufs=2, space="PSUM"))
        ps_pv = ctx.enter_context(tc.tile_pool(name="ps_pv", bufs=3, space="PSUM"))

        def psA():
            return ps_a.tile([128, 512], F32, tag="A", name="pA")

        def psB():
            return ps_b.tile([128, 512], F32, tag="B", name="pB")

        def psT():
            return ps_b.tile([128, 512], F32, tag="B", name="pT")

        def wstage():
            return stg.tile([128, 512], F32, tag="wstage", name="wstg")

        ident = sb.tile([128, 128], F32, tag="ident")
        make_identity(nc, ident[:])
        ident_b = sb.tile([128, 128], BF16, tag="ident_b")
        nc.vector.tensor_copy(ident_b[:], ident[:])
        onesf = sb.tile([128, 1], F32, tag="onesf")
        nc.vector.memset(onesf[:], 1.0)

        # binary causal mask (keep q >= k within the diagonal chunk)
        mask_f = wstage()
        nc.vector.memset(mask_f[:], 1.0)
        nc.gpsimd.affine_select(out=mask_f[:], in_=mask_f[:],
                                compare_op=OP.is_ge, fill=0.0, base=0,
                                channel_multiplier=-1, pattern=[[1, 512]])
        mask_b = sb.tile([128, 512], BF16, tag="mask_b")
        nc.vector.tensor_copy(mask_b[:], mask_f[:])

        # column sums for the norm affine fixes
        wsum_qc = sb.tile([128, DT], F32, tag="wsum_qc")
        wsum_f1 = sb.tile([128, FT], F32, tag="wsum_f1")

        # ---- resident transposed weights (bf16) ----
        wT = {w: sb.tile([128, DT, D], BF16, tag=f"T_{w}", name=f"T_{w}")
              for w in WNAMES}
        wf1T = sb.tile([128, DT, FF], BF16, tag="T_wf1")
        wf2T = sb.tile([128, FT, D], BF16, tag="T_wf2")

        # ---- activations ----
        x_T = sb.tile([128, DT, S], BF16, tag="g_x")
        enc_T = sb.tile([128, DT, S], BF16, tag="g_enc")
        q_T = sb.tile([128, DT, S], BF16, tag="g_q")
        k_T = sb.tile([128, DT, S], BF16, tag="g_k")
        v_tok = sb.tile([128, TT, H * 65], BF16, tag="g_v")
        attn_T = sb.tile([128, DT, S], BF16, tag="g_attn")
        r1_T = sb.tile([128, DT, S], BF16, tag="g_r1")
        r2_T = sb.tile([128, DT, S], BF16, tag="g_r2")
        h_T = sb.tile([128, FT, S], BF16, tag="g_h")
        r3_T = sb.tile([128, DT, S], BF16, tag="g_r3")
        r3_tok = sb.tile([128, TT, D], BF16, tag="g_r3tok")
        scr = sb.tile([128, 512], F32, tag="scr")

        # ---- transpose helpers ----
        def transpose_group4(dst_view, stage, wsum_col=None, engine="vector"):
            """stage [128(rows),512(=4x128 cols)] -> 4 transposed blocks into
            one psum bank, one grouped copy-out to dst_view
            ([128, 4, 128] view of a bf16 resident tile)."""
            pt = psT()
            for ki in range(4):
                nc.tensor.transpose(pt[:, ki * 128:(ki + 1) * 128],
                                    stage[:, ki * 128:(ki + 1) * 128], ident[:])
            pv_ = pt[:].rearrange("p (k c) -> p k c", c=128)
            if engine == "scalar":
                nc.scalar.copy(dst_view, pv_)
            else:
                nc.vector.tensor_copy(dst_view, pv_)
            if wsum_col is not None:
                nc.vector.reduce_sum(wsum_col, stage[:],
                                     axis=mybir.AxisListType.X)

        def stage_dma(src_ap):
            stage = wstage()
            nc.sync.dma_start(stage[:], src_ap)
            return stage

        # background work queue: each closure emits one stage of weight prep
        bg = []

        def pump(n):
            for _ in range(min(n, len(bg))):
                bg.pop(0)()

        def prep_w_steps(wname, dst, wsum=None):
            """[512,512] weight -> dst [128, DT, 512] transposed bf16."""
            for ot in range(DT):
                def step(ot=ot):
                    stage = stage_dma(
                        inp[wname].rearrange("(t p) i -> p t i", p=128)[:, ot])
                    wcol = wsum[:, ot:ot + 1] if wsum is not None else None
                    transpose_group4(dst[:, :, ot * 128:(ot + 1) * 128],
                                     stage, wcol)
                bg.append(step)

        def prep_wf1_steps():
            for ot in range(FT):
                def step(ot=ot):
                    stage = stage_dma(
                        inp["wf1"].rearrange("(t p) i -> p t i", p=128)[:, ot])
                    transpose_group4(wf1T[:, :, ot * 128:(ot + 1) * 128],
                                     stage, wsum_f1[:, ot:ot + 1])
                bg.append(step)

        def prep_wf2_steps():
            for dd in range(DT):
                for piece in range(4):
                    def step(dd=dd, piece=piece):
                        stage = stage_dma(
                            inp["wf2"].rearrange("(t p) i -> p t i", p=128)
                            [:, dd, piece * 512:(piece + 1) * 512])
                        transpose_group4(
                            wf2T[:, piece * 4:piece * 4 + 4,
                                 dd * 128:(dd + 1) * 128], stage)
                    bg.append(step)

        def prep_act_steps(src_d, dst_T, engine="vector"):
            """[S,D] activation -> dst_T [128, DT, S] bf16 feature-major."""
            for tt in range(TT):
                def step(tt=tt):
                    stage = stage_dma(
                        src_d.rearrange("(tt p) d -> p tt d", p=128)[:, tt])
                    transpose_group4(dst_T[:, :, tt * 128:(tt + 1) * 128],
                                     stage, engine=engine)
                bg.append(step)

        # ---- projection helpers ----
        def project_fm(w, src_T, out_tile, bias_tile=None, out_engine="vector"):
            """Feature-major projection: out[:, dd, :] = W^T-block @ src.
            bias_tile=None leaves the output raw (bias/norm fix applied later
            in-place, so an AllReduce wait never backs up the PSUM banks)."""
            for dd in range(DT):
                for th in range(TH):
                    pt = psB()
                    for ki in range(DT):
                        nc.tensor.matmul(pt[:], wT[w][:, ki, dd * 128:(dd + 1) * 128],
                                         src_T[:, ki, th * 512:(th + 1) * 512],
                                         start=(ki == 0), stop=(ki == DT - 1))
                    dst = out_tile[:, dd, th * 512:(th + 1) * 512]
                    if out_engine == "scalar":
                        if bias_tile is None:
                            nc.scalar.copy(dst, pt[:])
                        else:
                            nc.scalar.activation(dst, pt[:], AF.Identity,
                                                 bias=bias_tile[:, dd:dd + 1])
                    else:
                        if bias_tile is None:
                            nc.vector.tensor_copy(dst, pt[:])
                        else:
                            nc.vector.tensor_scalar(dst, pt[:],
                                                    bias_tile[:, dd:dd + 1],
                                                    None, OP.add)

        def project_v(w, bname, src_T):
            """Token-major v with per-head ones column: v_tok [128,TT,H*65]."""
            ones_view = v_tok[:, :, :].rearrange(
                "p t (h c) -> p t h c", c=65)[:, :, :, 64]
            nc.vector.tensor_copy(
                ones_view, onesf[:, 0:1, None].to_broadcast([128, TT, H]))
            for tt in range(TT):
                pt = psB()
                for ki in range(DT):
                    nc.tensor.matmul(pt[:], src_T[:, ki, tt * 128:(tt + 1) * 128],
                                     wT[w][:, ki],
                                     start=(ki == 0), stop=(ki == DT - 1))
                dstv = v_tok[:, tt].rearrange("p (h c) -> p h c", c=65)[:, :, 0:64]
                nc.vector.tensor_tensor(
                    dstv, pt[:].rearrange("p (h c) -> p h c", c=64),
                    bv_full[bname][:].rearrange("p (h c) -> p h c", c=64),
                    OP.add)

        def attention(q_t, k_t, attn_t, causal, pump_n=0):
            for h in range(H):
                dt_, base = h // 2, (h % 2) * 64
                q_h = q_t[base:base + 64, dt_]
                k_h = k_t[base:base + 64, dt_]
                pv = {qh: ps_pv.tile([128, 512], F32, tag="PV", name="pPV")
                      for qh in range(TH)}
                for kt in range(TT):
                    pr = cp.tile([128, S], BF16, tag="probs", name="probs", bufs=3)
                    if causal:
                        m0, mw, clean = CAUSAL_CHUNKS[kt]
                        chunks = [(m0, mw, True)] + [(c0, cw, False)
                                                     for (c0, cw) in clean]
                    else:
                        chunks = [(0, 512, False), (512, 512, False)]
                    for (c0, cw, masked) in chunks:
                        st = psA()
                        nc.tensor.matmul(st[:, :cw],
                                         k_h[:, kt * 128:(kt + 1) * 128],
                                         q_h[:, c0:c0 + cw],
                                         start=True, stop=True)
                        nc.scalar.activation(pr[:, c0:c0 + cw], st[:, :cw],
                                             AF.Exp, scale=1.0 / 32.0)
                        if masked:
                            # only the first 128 cols are the triangle block
                            nc.gpsimd.affine_select(
                                out=pr[:, c0:c0 + 128], in_=pr[:, c0:c0 + 128],
                                compare_op=OP.is_ge, fill=0.0, base=0,
                                channel_multiplier=-1, pattern=[[1, 128]])
                    # PV contributions of this kt
                    v_h = v_tok[:, kt, h * 65:(h + 1) * 65]
                    for qh in range(TH):
                        if causal and qh == 0 and kt > 3:
                            continue
                        if causal:
                            off = max(0, (kt - qh * 4) * 128)
                            last = (kt == 3) if qh == 0 else (kt == 7)
                        else:
                            off, last = 0, (kt == 7)
                        nc.tensor.matmul(
                            pv[qh][:65, off:512], v_h,
                            pr[:, qh * 512 + off:(qh + 1) * 512],
                            start=(kt == 0), stop=last)
                # normalize by the rowsum living in row 64 of pv
                for qh in range(TH):
                    rec = cp.tile([1, 512], F32, tag="rsrec", name="rec", bufs=1)
                    nc.vector.reciprocal(rec[:], pv[qh][64:65, :])
                    rb = cp.tile([64, 512], F32, tag="rsbc", name="rb")
                    nc.gpsimd.partition_broadcast(rb[:], rec[:])
                    nc.vector.tensor_tensor(
                        attn_t[base:base + 64, dt_, qh * 512:(qh + 1) * 512],
                        pv[qh][0:64, :], rb[:], OP.mult)
                if pump_n:
                    pump(pump_n)

        def residual_out(w, src_T, bias_tile, res_T, out_T, stats_sb):
            """out_T = (W^T @ src_T) + bias + res_T ; accumulate sum/sumsq."""
            n_ki = src_T.shape[1]
            for dd in range(DT):
                for th in range(TH):
                    pt = psB()
                    for ki in range(n_ki):
                        nc.tensor.matmul(pt[:], wT[w][:, ki, dd * 128:(dd + 1) * 128],
                                         src_T[:, ki, th * 512:(th + 1) * 512],
                                         start=(ki == 0), stop=(ki == n_ki - 1))
                    dst = out_T[:, dd, th * 512:(th + 1) * 512]
                    c = dd * TH + th
                    nc.vector.scalar_tensor_tensor(
                        dst, pt[:], bias_tile[:, dd:dd + 1],
                        res_T[:, dd, th * 512:(th + 1) * 512],
                        OP.add, OP.add, accum_out=stats_sb[:, c:c + 1])
                    nc.vector.scalar_tensor_tensor(
                        scr[:], dst, 0.0, dst, OP.add, OP.mult,
                        accum_out=stats_sb[:, 8 + c:8 + c + 1])

        def stats_ar_kick(stats_sb, name):
            pt = psA()
            nc.tensor.matmul(pt[:1, :16], onesf[:], stats_sb[:],
                             start=True, stop=True)
            red = sb.tile([1, 8], F32, tag=f"red_{name}")
            nc.vector.reduce_sum(red[:, 0:1], pt[0:1, 0:8],
                                 axis=mybir.AxisListType.X)
            nc.vector.reduce_sum(red[:, 1:2], pt[0:1, 8:16],
                                 axis=mybir.AxisListType.X)
            nc.vector.memset(red[:, 2:8], 0.0)
            ar_in = dram.tile([1, 8], F32, tag=f"ar_in_{name}")
            ar_out = dram.tile([1, 8], F32, tag=f"ar_out_{name}")
            nc.gpsimd.dma_start(ar_in[:], red[:])
            nc.gpsimd.collective_compute(
                "AllReduce", OP.add, replica_groups=[list(range(N_CORES))],
                ins=[ar_in.opt()], outs=[ar_out.opt()])
            g = sb.tile([1, 8], F32, tag=f"g_{name}")
            nc.sync.dma_start(g[:], ar_out[:])
            return g

        def stats_ar_finish(g, name):
            mu = sb.tile([1, 1], F32, tag=f"mu_{name}")
            nc.vector.tensor_scalar_mul(mu[:], g[:, 0:1], 1.0 / NTOT)
            ex2 = sb.tile([1, 1], F32, tag=f"ex2_{name}")
            nc.vector.tensor_scalar_mul(ex2[:], g[:, 1:2], 1.0 / NTOT)
            mu2 = sb.tile([1, 1], F32, tag=f"mu2_{name}")
            nc.vector.tensor_tensor(mu2[:], mu[:], mu[:], OP.mult)
            var = sb.tile([1, 1], F32, tag=f"var_{name}")
            nc.vector.tensor_tensor(var[:], ex2[:], mu2[:], OP.subtract)
            epst = sb.tile([1, 1], F32, tag=f"eps_{name}")
            nc.vector.memset(epst[:], EPS)
            std = sb.tile([1, 1], F32, tag=f"std_{name}")
            nc.scalar.activation(std[:], var[:], AF.Sqrt, bias=epst[:])
            rstd = sb.tile([1, 1], F32, tag=f"rstd_{name}")
            nc.vector.reciprocal(rstd[:], std[:])
            nmr = sb.tile([1, 1], F32, tag=f"nmr_{name}")
            nc.vector.tensor_tensor(nmr[:], mu[:], rstd[:], OP.mult)
            nc.vector.tensor_scalar_mul(nmr[:], nmr[:], -1.0)
            rstd_bc = sb.tile([128, 1], F32, tag=f"rstd_bc_{name}")
            nc.gpsimd.partition_broadcast(rstd_bc[:], rstd[:])
            nmr_bc = sb.tile([128, 1], F32, tag=f"nmr_bc_{name}")
            nc.gpsimd.partition_broadcast(nmr_bc[:], nmr[:])
            return rstd_bc, nmr_bc

        def materialize_norm(src_T, dst_T, rstd_bc, nmr_bc):
            for dd in range(DT):
                nc.vector.scalar_tensor_tensor(
                    dst_T[:, dd], src_T[:, dd], rstd_bc[:],
                    nmr_bc[:, :].to_broadcast([128, S]), OP.mult, OP.add)

        # ================= Phase 0: staged loads =================
        # x + self-attn weights emitted inline (they gate phase 1);
        # later weights go on the background queue, pumped between heads.
        prep_act_steps(inp["data_dec"], x_T, engine="scalar")
        prep_w_steps("wq_m", wT["wq_m"])
        prep_w_steps("wk_m", wT["wk_m"])
        prep_w_steps("wv_m", wT["wv_m"])
        pump(len(bg))  # emit now: x, wq, wk, wv
        # ---- biases (f32, used as per-partition scalar operands) ----
        bias = {}
        for b in BNAMES + ["bf2"]:
            t = sb.tile([128, DT], F32, tag=f"{b}_sb")
            nc.sync.dma_start(t[:], inp[b].rearrange("(t p) -> p t", p=128))
            bias[b] = t
        bf1_sb = sb.tile([128, FT], F32, tag="bf1_sb")
        nc.sync.dma_start(bf1_sb[:], inp["bf1"].rearrange("(t p) -> p t", p=128))
        bv_full = {}
        for b in ("bv_m", "bv_c"):
            row = wstage()
            nc.sync.dma_start(row[0:1, :], inp[b][None, :])
            full = sb.tile([128, D], F32, tag=f"{b}_full")
            nc.gpsimd.partition_broadcast(full[:], row[0:1, :])
            bv_full[b] = full

        prep_w_steps("wo_m", wT["wo_m"])
        pump(2)

        project_fm("wq_m", x_T, q_T, bias_tile=bias["bq_m"])
        project_fm("wk_m", x_T, k_T, bias_tile=bias["bk_m"],
                   out_engine="scalar")
        project_v("wv_m", "bv_m", x_T)
        pump(len(bg))  # rest of wo_m

        # warm up the collective stream so AllReduce #1 is not the first op
        ar_wi = dram.tile([1, 8], F32, tag="ar_wi")
        ar_wo = dram.tile([1, 8], F32, tag="ar_wo")
        warm8 = sb.tile([1, 8], F32, tag="warm8")
        nc.vector.memset(warm8[:], 0.0)
        nc.gpsimd.dma_start(ar_wi[:], warm8[:])
        nc.gpsimd.collective_compute(
            "AllReduce", OP.add, replica_groups=[list(range(N_CORES))],
            ins=[ar_wi.opt()], outs=[ar_wo.opt()])

        # queue cross-attn weights + enc for pumping inside self-attention
        prep_act_steps(inp["encoder_out"], enc_T)
        prep_w_steps("wk_c", wT["wk_c"])
        prep_w_steps("wv_c", wT["wv_c"])
        prep_w_steps("wq_c", wT["wq_c"], wsum=wsum_qc)
        prep_w_steps("wo_c", wT["wo_c"])

        # ================= Phase 1: self attention =================
        attention(q_T, k_T, attn_T, causal=True, pump_n=3)
        pump(len(bg))

        stats1 = sb.tile([128, 16], F32, tag="stats1")
        residual_out("wo_m", attn_T, bias["bo_m"], x_T, r1_T, stats1)
        g1 = stats_ar_kick(stats1, "n1")

        # ================= Phase 2: cross attention =================
        # k/v/q-raw projections (no AR dependency) overlap the AllReduce;
        # q's norm fix lands in-place afterwards so PSUM never backs up.
        project_fm("wk_c", enc_T, k_T, bias_tile=bias["bk_c"])
        project_v("wv_c", "bv_c", enc_T)
        project_fm("wq_c", r1_T, q_T, bias_tile=None, out_engine="scalar")
        rstd1, nmr1 = stats_ar_finish(g1, "n1")
        qfix = sb.tile([128, DT], F32, tag="qfix")
        for dd in range(DT):
            nc.vector.scalar_tensor_tensor(
                qfix[:, dd:dd + 1], wsum_qc[:, dd:dd + 1], nmr1[:],
                bias["bq_c"][:, dd:dd + 1], OP.mult, OP.add)
        for dd in range(DT):
            nc.vector.tensor_scalar(q_T[:, dd], q_T[:, dd], rstd1[:],
                                    qfix[:, dd:dd + 1], OP.mult, OP.add)

        # queue FFN weights; a few pumps fill the AR1/q-fix wait
        prep_wf1_steps()
        prep_wf2_steps()
        pump(6)

        attention(q_T, k_T, attn_T, causal=False, pump_n=4)
        pump(len(bg))

        nmm_T = sb.tile([128, DT, S], BF16, tag="g_x")     # reuses x_T space
        materialize_norm(r1_T, nmm_T, rstd1, nmr1)
        stats2 = sb.tile([128, 16], F32, tag="stats2")
        residual_out("wo_c", attn_T, bias["bo_c"], nmm_T, r2_T, stats2)
        g2 = stats_ar_kick(stats2, "n2")

        # ================= Phase 3: FFN =================
        # ff1 raw matmuls for BOTH halves run during the AllReduce; the
        # relu+scale pass (AR-dependent) is applied in-place on h_T.
        for th in range(TH):
            for ft in range(FT):
                pt = psB()
                for ki in range(DT):
                    nc.tensor.matmul(pt[:], wf1T[:, ki, ft * 128:(ft + 1) * 128],
                                     r2_T[:, ki, th * 512:(th + 1) * 512],
                                     start=(ki == 0), stop=(ki == DT - 1))
                nc.vector.tensor_copy(h_T[:, ft, th * 512:(th + 1) * 512],
                                      pt[:])
        rstd2, nmr2 = stats_ar_finish(g2, "n2")
        ffix = sb.tile([128, FT], F32, tag="ffix")
        for ft in range(FT):
            nc.vector.scalar_tensor_tensor(
                ffix[:, ft:ft + 1], wsum_f1[:, ft:ft + 1], nmr2[:],
                bf1_sb[:, ft:ft + 1], OP.mult, OP.add)
        for ft in range(FT):
            if ft % 2 == 0:
                nc.scalar.activation(h_T[:, ft], h_T[:, ft], AF.Relu,
                                     bias=ffix[:, ft:ft + 1], scale=rstd2[:])
            else:
                nc.vector.tensor_scalar(h_T[:, ft], h_T[:, ft], rstd2[:],
                                        ffix[:, ft:ft + 1], OP.mult, OP.add)
                nc.vector.tensor_scalar_max(h_T[:, ft], h_T[:, ft], 0.0)
        nmh_T = sb.tile([128, DT, S], BF16, tag="g_enc")   # reuses enc_T space
        materialize_norm(r2_T, nmh_T, rstd2, nmr2)

        stats3 = sb.tile([128, 16], F32, tag="stats3")

        def r3_transpose(tt):
            pt = ps_b.tile([128, 512], BF16, tag="B", name="pT3")
            for dd in range(DT):
                nc.tensor.transpose(pt[:, dd * 128:(dd + 1) * 128],
                                    r3_T[:, dd, tt * 128:(tt + 1) * 128],
                                    ident_b[:])
            nc.vector.tensor_copy(
                r3_tok[:, tt].rearrange("p (k c) -> p k c", c=128),
                pt[:].rearrange("p (k c) -> p k c", c=128))

        for th in range(TH):
            for dd in range(DT):
                pt = psB()
                for ki in range(FT):
                    nc.tensor.matmul(pt[:], wf2T[:, ki, dd * 128:(dd + 1) * 128],
                                     h_T[:, ki, th * 512:(th + 1) * 512],
                                     start=(ki == 0), stop=(ki == FT - 1))
                dst = r3_T[:, dd, th * 512:(th + 1) * 512]
                c = dd * TH + th
                nc.vector.scalar_tensor_tensor(
                    dst, pt[:], bias["bf2"][:, dd:dd + 1],
                    nmh_T[:, dd, th * 512:(th + 1) * 512], OP.add, OP.add,
                    accum_out=stats3[:, c:c + 1])
                nc.vector.scalar_tensor_tensor(
                    scr[:], dst, 0.0, dst, OP.add, OP.mult,
                    accum_out=stats3[:, 8 + c:8 + c + 1])
            # r3 token-tiles of this half are complete: transpose them now
            # (fills PE gaps while ff2's copy-outs drain / AR3 later)
            for tt in range(th * 4, th * 4 + 4):
                r3_transpose(tt)
        g3 = stats_ar_kick(stats3, "n3")
        rstd3, nmr3 = stats_ar_finish(g3, "n3")
        for tt in range(TT):
            ost = wstage()
            nc.vector.scalar_tensor_tensor(
                ost[:], r3_tok[:, tt], rstd3[:],
                nmr3[:, :].to_broadcast([128, D]), OP.mult, OP.add)
            nc.sync.dma_start(
                out_d.rearrange("(tt p) d -> p tt d", p=128)[:, tt],
                ost[:])


_NC_CACHE = {}


def kernel(**inputs):
    if "nc" not in _NC_CACHE:
        _NC_CACHE["nc"] = build_nc()
    nc = _NC_CACHE["nc"]
    in_maps = []
    for b in range(N_CORES):
        m = {"data_dec": np.ascontiguousarray(
                 np.asarray(inputs["data_dec"], dtype=np.float32)[b]),
             "encoder_out": np.ascontiguousarray(
                 np.asarray(inputs["encoder_out"], dtype=np.float32)[b])}
        for k, v in inputs.items():
            if k not in ("data_dec", "encoder_out"):
                m[k] = np.ascontiguousarray(np.asarray(v, dtype=np.float32))
        in_maps.append(m)
    res = bass_utils.run_bass_kernel_spmd(nc, in_maps,
                                          core_ids=list(range(N_CORES)))
    return np.stack([res.results[b]["out"] for b in range(N_CORES)], axis=0)


# revision 15
# speedup vs baseline: 1.0640x; 1.0239x over previous
"""Trainium2 Bass kernel for nn_Decoder_Model (dense transformer decoder layer).

Sharding: data-parallel over batch (8 batches -> 8 cores). The three global
layernorms (normalized over ALL elements of the [B,S,D] tensor) need cross-core
scalar stats: each core computes local sum/sumsq, an 8-float AllReduce merges
them. AllReduce latency is hidden behind the next phase's matmuls using the
affine trick: norm(x)@W.T = (x@W.T)*rstd + per-channel-fix, so the big matmuls
run on raw x while stats are in flight and only a cheap fixup pass waits.

Perf notes vs the fp32r baseline:
- All matmul operands are bf16 (PSUM stays fp32): halves PE weight-load time,
  SBUF traffic and weight DMA; tolerance is 2e-2 so ~0.5% rounding is fine.
- Weights are transposed ONCE into resident SBUF (bf16), with psum->sbuf
  copy-outs grouped [128,512] and issued on gpsimd (scalar engine is reserved
  for softmax exp, its hard floor).
- Weight prep for later phases is pumped between attention heads so the PE
  never idles long enough to re-engage the HAM half-rate throttle.
- Cross-attn k/v projections are emitted before the q fix so AllReduce #1
  latency hides behind them.
"""
import sys

import numpy as np

sys.path.insert(0, "/opt/trn_rl_repo")

import concourse.bass as bass  # noqa: E402,F401
import concourse.mybir as mybir  # noqa: E402
import concourse.tile as tile  # noqa: E402
from concourse import bacc  # noqa: E402
from concourse import bass_utils  # noqa: E402
from concourse.masks import make_identity  # noqa: E402

F32 = mybir.dt.float32
F32R = mybir.dt.float32r
BF16 = mybir.dt.bfloat16
AF = mybir.ActivationFunctionType
OP = mybir.AluOpType

B, S, D, H, DK, FF = 8, 1024, 512, 8, 64, 2048
TT = S // 128   # 8 token tiles
DT = D // 128   # 4 feature tiles
FT = FF // 128  # 16 ffn tiles
TH = S // 512   # 2 matmul free-dim halves
N_CORES = 8
NTOT = float(B * S * D)
EPS = 1e-5

WNAMES = ["wq_m", "wk_m", "wv_m", "wo_m", "wq_c", "wk_c", "wv_c", "wo_c"]
BNAMES = ["bq_m", "bk_m", "bv_m", "bo_m", "bq_c", "bk_c", "bv_c", "bo_c"]

# self-attn causal chunking per k-tile kt over the q axis:
# (masked_chunk_start, masked_chunk_width, [(clean_start, clean_width), ...])
CAUSAL_CHUNKS = {
    0: (0, 256, [(256, 512), (768, 256)]),
    1: (128, 256, [(384, 384), (768, 256)]),
    2: (256, 256, [(512, 512)]),
    3: (384, 256, [(640, 384)]),
    4: (512, 256, [(768, 256)]),
    5: (640, 384, []),
    6: (768, 256, []),
    7: (896, 128, []),
}


def build_nc():
    nc = bacc.Bacc("TRN2", target_bir_lowering=False, debug=False,
                   enable_asserts=False, num_devices=N_CORES)
    inp = {}
    inp["data_dec"] = nc.dram_tensor("data_dec", [S, D], F32,
                                     kind="ExternalInput").ap()
    inp["encoder_out"] = nc.dram_tensor("encoder_out", [S, D], F32,
                                        kind="ExternalInput").ap()
    for w in WNAMES:
        inp[w] = nc.dram_tensor(w, [D, D], F32, kind="ExternalInput").ap()
    for b in BNAMES:
        inp[b] = nc.dram_tensor(b, [D], F32, kind="ExternalInput").ap()
    inp["wf1"] = nc.dram_tensor("wf1", [FF, D], F32, kind="ExternalInput").ap()
    inp["bf1"] = nc.dram_tensor("bf1", [FF], F32, kind="ExternalInput").ap()
    inp["wf2"] = nc.dram_tensor("wf2", [D, FF], F32, kind="ExternalInput").ap()
    inp["bf2"] = nc.dram_tensor("bf2", [D], F32, kind="ExternalInput").ap()
    out_d = nc.dram_tensor("out", [S, D], F32, kind="ExternalOutput").ap()

    with tile.TileContext(nc) as tc:
        build_body(nc, tc, inp, out_d)
    nc.finalize()
    return nc


def build_body(nc, tc, inp, out_d):
    import contextlib
    ctx = contextlib.ExitStack()
    with ctx:
        sb = ctx.enter_context(tc.tile_pool(name="sb", bufs=1))
        stg = ctx.enter_context(tc.tile_pool(name="stg", bufs=4))
        cp = ctx.enter_context(tc.tile_pool(name="cp", bufs=2))
        dram = ctx.enter_context(tc.tile_pool(name="dram", bufs=1, space="DRAM"))
        ps_a = ctx.enter_context(tc.tile_pool(name="ps_a", bufs=2, space="PSUM"))
        ps_b = ctx.enter_context(tc.tile_pool(name="ps_b", bufs=3, space="PSUM"))
        ps_pv = ctx.enter_context(tc.tile_pool(name="ps_pv", bufs=3, space="PSUM"))

        def psA():
            return ps_a.tile([128, 512], F32, tag="A", name="pA")

        def psB():
            return ps_b.tile([128, 512], F32, tag="B", name="pB")

        def psT():
            return ps_b.tile([128, 512], F32, tag="B", name="pT")

        def wstage():
            return stg.tile([128, 512], F32, tag="wstage", name="wstg")

        ident = sb.tile([128, 128], F32, tag="ident")
        make_identity(nc, ident[:])
        ident_b = sb.tile([128, 128], BF16, tag="ident_b")
        nc.vector.tensor_copy(ident_b[:], ident[:])
        onesf = sb.tile([128, 1], F32, tag="onesf")
        nc.vector.memset(onesf[:], 1.0)

        # column sums for the norm affine fixes
        wsum_qc = sb.tile([128, DT], F32, tag="wsum_qc")
        wsum_f1 = sb.tile([128, FT], F32, tag="wsum_f1")

        # ---- resident transposed weights (bf16) ----
        wT = {w: sb.tile([128, DT, D], BF16, tag=f"T_{w}", name=f"T_{w}")
              for w in WNAMES}
        wf1T = sb.tile([128, DT, FF], BF16, tag="T_wf1")
        wf2T = sb.tile([128, FT, D], BF16, tag="T_wf2")

        # ---- activations ----
        x_T = sb.tile([128, DT, S], BF16, tag="g_x")
        enc_T = sb.tile([128, DT, S], BF16, tag="g_enc")
        q_T = sb.tile([128, DT, S], BF16, tag="g_q")
        k_T = sb.tile([128, DT, S], BF16, tag="g_k")
        v_tok = sb.tile([128, TT, H * 65], BF16, tag="g_v")
        attn_T = sb.tile([128, DT, S], BF16, tag="g_attn")
        r1_T = sb.tile([128, DT, S], BF16, tag="g_r1")
        r2_T = sb.tile([128, DT, S], BF16, tag="g_r2")
        h_T = sb.tile([128, FT, S], BF16, tag="g_h")
        r3_T = sb.tile([128, DT, S], BF16, tag="g_r3")
        r3_tok = sb.tile([128, TT, D], BF16, tag="g_r3tok")
        scr = sb.tile([128, 512], F32, tag="scr")

        # ---- transpose helpers ----
        def transpose_group4(dst_view, stage, wsum_col=None, engine="vector"):
            """stage [128(rows),512(=4x128 cols)] -> 4 transposed blocks into
            one psum bank, one grouped copy-out to dst_view
            ([128, 4, 128] view of a bf16 resident tile)."""
            pt = psT()
            for ki in range(4):
                nc.tensor.transpose(pt[:, ki * 128:(ki + 1) * 128],
                                    stage[:, ki * 128:(ki + 1) * 128], ident[:])
            pv_ = pt[:].rearrange("p (k c) -> p k c", c=128)
            if engine == "scalar":
                nc.scalar.copy(dst_view, pv_)
            else:
                nc.vector.tensor_copy(dst_view, pv_)
            if wsum_col is not None:
                nc.vector.reduce_sum(wsum_col, stage[:],
                                     axis=mybir.AxisListType.X)

        def stage_dma(src_ap):
            stage = wstage()
            nc.sync.dma_start(stage[:], src_ap)
            return stage

        # background work queue: each closure emits one stage of weight prep
        bg = []

        def pump(n):
            for _ in range(min(n, len(bg))):
                bg.pop(0)()

        def prep_w_steps(wname, dst, wsum=None):
            """[512,512] weight -> dst [128, DT, 512] transposed bf16."""
            for ot in range(DT):
                def step(ot=ot):
                    stage = stage_dma(
                        inp[wname].rearrange("(t p) i -> p t i", p=128)[:, ot])
                    wcol = wsum[:, ot:ot + 1] if wsum is not None else None
                    transpose_group4(dst[:, :, ot * 128:(ot + 1) * 128],
                                     stage, wcol)
                bg.append(step)

        def prep_wf1_steps():
            for ot in range(FT):
                def step(ot=ot):
                    stage = stage_dma(
                        inp["wf1"].rearrange("(t p) i -> p t i", p=128)[:, ot])
                    transpose_group4(wf1T[:, :, ot * 128:(ot + 1) * 128],
                                     stage, wsum_f1[:, ot:ot + 1])
                bg.append(step)

        def prep_wf2_steps():
            for dd in range(DT):
                for piece in range(4):
                    def step(dd=dd, piece=piece):
                        stage = stage_dma(
                            inp["wf2"].rearrange("(t p) i -> p t i", p=128)
                            [:, dd, piece * 512:(piece + 1) * 512])
                        transpose_group4(
                            wf2T[:, piece * 4:piece * 4 + 4,
                                 dd * 128:(dd + 1) * 128], stage)
                    bg.append(step)

        def prep_act_steps(src_d, dst_T, engine="vector"):
            """[S,D] activation -> dst_T [128, DT, S] bf16 feature-major."""
            for tt in range(TT):
                def step(tt=tt):
                    stage = stage_dma(
                        src_d.rearrange("(tt p) d -> p tt d", p=128)[:, tt])
                    transpose_group4(dst_T[:, :, tt * 128:(tt + 1) * 128],
                                     stage, engine=engine)
                bg.append(step)

        # ---- projection helpers ----
        def project_fm(w, src_T, out_tile, bias_tile=None, out_engine="vector"):
            """Feature-major projection: out[:, dd, :] = W^T-block @ src.
            bias_tile=None leaves the output raw (bias/norm fix applied later
            in-place, so an AllReduce wait never backs up the PSUM banks)."""
            for dd in range(DT):
                for th in range(TH):
                    pt = psB()
                    for ki in range(DT):
                        nc.tensor.matmul(pt[:], wT[w][:, ki, dd * 128:(dd + 1) * 128],
                                         src_T[:, ki, th * 512:(th + 1) * 512],
                                         start=(ki == 0), stop=(ki == DT - 1))
                    dst = out_tile[:, dd, th * 512:(th + 1) * 512]
                    if out_engine == "scalar":
                        if bias_tile is None:
                            nc.scalar.copy(dst, pt[:])
                        else:
                            nc.scalar.activation(dst, pt[:], AF.Identity,
                                                 bias=bias_tile[:, dd:dd + 1])
                    else:
                        if bias_tile is None:
                            nc.vector.tensor_copy(dst, pt[:])
                        else:
                            nc.vector.tensor_scalar(dst, pt[:],
                                                    bias_tile[:, dd:dd + 1],
                                                    None, OP.add)

        def project_v(w, bname, src_T):
            """Token-major v with per-head ones column: v_tok [128,TT,H*65]."""
            ones_view = v_tok[:, :, :].rearrange(
                "p t (h c) -> p t h c", c=65)[:, :, :, 64]
            nc.vector.tensor_copy(
                ones_view, onesf[:, 0:1, None].to_broadcast([128, TT, H]))
            for tt in range(TT):
                pt = psB()
                for ki in range(DT):
                    nc.tensor.matmul(pt[:], src_T[:, ki, tt * 128:(tt + 1) * 128],
                                     wT[w][:, ki],
                                     start=(ki == 0), stop=(ki == DT - 1))
                dstv = v_tok[:, tt].rearrange("p (h c) -> p h c", c=65)[:, :, 0:64]
                nc.vector.tensor_tensor(
                    dstv, pt[:].rearrange("p (h c) -> p h c", c=64),
                    bv_full[bname][:].rearrange("p (h c) -> p h c", c=64),
                    OP.add)

        def attention(q_t, k_t, attn_t, causal, pump_n=0):
            for h in range(H):
                dt_, base = h // 2, (h % 2) * 64
                q_h = q_t[base:base + 64, dt_]
                k_h = k_t[base:base + 64, dt_]
                pv = {qh: ps_pv.tile([128, 512], F32, tag="PV", name="pPV")
                      for qh in range(TH)}
                for kt in range(TT):
                    pr = cp.tile([128, S], BF16, tag="probs", name="probs")
                    if causal:
                        m0, mw, clean = CAUSAL_CHUNKS[kt]
                        chunks = [(m0, mw, True)] + [(c0, cw, False)
                                                     for (c0, cw) in clean]
                    else:
                        chunks = [(0, 512, False), (512, 512, False)]
                    for (c0, cw, masked) in chunks:
                        st = psA()
                        nc.tensor.matmul(st[:, :cw],
                                         k_h[:, kt * 128:(kt + 1) * 128],
                                         q_h[:, c0:c0 + cw],
                                         start=True, stop=True)
                        nc.scalar.activation(pr[:, c0:c0 + cw], st[:, :cw],
                                             AF.Exp, scale=1.0 / 32.0)
                        if masked:
                            # only the first 128 cols are the triangle block
                            nc.gpsimd.affine_select(
                                out=pr[:, c0:c0 + 128], in_=pr[:, c0:c0 + 128],
                                compare_op=OP.is_ge, fill=0.0, base=0,
                                channel_multiplier=-1, pattern=[[1, 128]])
                    # PV contributions of this kt
                    v_h = v_tok[:, kt, h * 65:(h + 1) * 65]
                    for qh in range(TH):
                        if causal and qh == 0 and kt > 3:
                            continue
                        if causal:
                            off = max(0, (kt - qh * 4) * 128)
                            last = (kt == 3) if qh == 0 else (kt == 7)
                        else:
                            off, last = 0, (kt == 7)
                        nc.tensor.matmul(
                            pv[qh][:65, off:512], v_h,
                            pr[:, qh * 512 + off:(qh + 1) * 512],
                            start=(kt == 0), stop=last)
                # copy pv out of PSUM at once (frees the bank for the next
                # head), then normalize by the rowsum in row 64 from SBUF
                for qh in range(TH):
                    pvs = cp.tile([65, 512], F32, tag="pvstage", name="pvs")
                    nc.vector.tensor_copy(pvs[:], pv[qh][:65, :])
                    rec = cp.tile([1, 512], F32, tag="rsrec", name="rec", bufs=1)
                    nc.vector.reciprocal(rec[:], pvs[64:65, :])
                    rb = cp.tile([64, 512], F32, tag="rsbc", name="rb")
                    nc.gpsimd.partition_broadcast(rb[:], rec[:])
                    nc.vector.tensor_tensor(
                        attn_t[base:base + 64, dt_, qh * 512:(qh + 1) * 512],
                        pvs[0:64, :], rb[:], OP.mult)
                if pump_n:
                    pump(pump_n)

        def residual_out(w, src_T, bias_tile, res_T, out_T, stats_sb):
            """out_T = (W^T @ src_T) + bias + res_T ; accumulate sum/sumsq."""
            n_ki = src_T.shape[1]
            for dd in range(DT):
                for th in range(TH):
                    pt = psB()
                    for ki in range(n_ki):
                        nc.tensor.matmul(pt[:], wT[w][:, ki, dd * 128:(dd + 1) * 128],
                                         src_T[:, ki, th * 512:(th + 1) * 512],
                                         start=(ki == 0), stop=(ki == n_ki - 1))
                    dst = out_T[:, dd, th * 512:(th + 1) * 512]
                    c = dd * TH + th
                    nc.vector.scalar_tensor_tensor(
                        dst, pt[:], bias_tile[:, dd:dd + 1],
                        res_T[:, dd, th * 512:(th + 1) * 512],
                        OP.add, OP.add, accum_out=stats_sb[:, c:c + 1])
                    nc.vector.scalar_tensor_tensor(
                        scr[:], dst, 0.0, dst, OP.add, OP.mult,
                        accum_out=stats_sb[:, 8 + c:8 + c + 1])

        def stats_ar_kick(stats_sb, name):
            pt = psA()
            nc.tensor.matmul(pt[:1, :16], onesf[:], stats_sb[:],
                             start=True, stop=True)
            red = sb.tile([1, 8], F32, tag=f"red_{name}")
            nc.vector.reduce_sum(red[:, 0:1], pt[0:1, 0:8],
                                 axis=mybir.AxisListType.X)
            nc.vector.reduce_sum(red[:, 1:2], pt[0:1, 8:16],
                                 axis=mybir.AxisListType.X)
            nc.vector.memset(red[:, 2:8], 0.0)
            ar_in = dram.tile([1, 8], F32, tag=f"ar_in_{name}")
            ar_out = dram.tile([1, 8], F32, tag=f"ar_out_{name}")
            nc.gpsimd.dma_start(ar_in[:], red[:])
            nc.gpsimd.collective_compute(
                "AllReduce", OP.add, replica_groups=[list(range(N_CORES))],
                ins=[ar_in.opt()], outs=[ar_out.opt()])
            g = sb.tile([1, 8], F32, tag=f"g_{name}")
            nc.sync.dma_start(g[:], ar_out[:])
            return g

        def stats_ar_finish(g, name):
            mu = sb.tile([1, 1], F32, tag=f"mu_{name}")
            nc.vector.tensor_scalar_mul(mu[:], g[:, 0:1], 1.0 / NTOT)
            ex2 = sb.tile([1, 1], F32, tag=f"ex2_{name}")
            nc.vector.tensor_scalar_mul(ex2[:], g[:, 1:2], 1.0 / NTOT)
            mu2 = sb.tile([1, 1], F32, tag=f"mu2_{name}")
            nc.vector.tensor_tensor(mu2[:], mu[:], mu[:], OP.mult)
            var = sb.tile([1, 1], F32, tag=f"var_{name}")
            nc.vector.tensor_tensor(var[:], ex2[:], mu2[:], OP.subtract)
            epst = sb.tile([1, 1], F32, tag=f"eps_{name}")
            nc.vector.memset(epst[:], EPS)
            std = sb.tile([1, 1], F32, tag=f"std_{name}")
            nc.scalar.activation(std[:], var[:], AF.Sqrt, bias=epst[:])
            rstd = sb.tile([1, 1], F32, tag=f"rstd_{name}")
            nc.vector.reciprocal(rstd[:], std[:])
            nmr = sb.tile([1, 1], F32, tag=f"nmr_{name}")
            nc.vector.tensor_tensor(nmr[:], mu[:], rstd[:], OP.mult)
            nc.vector.tensor_scalar_mul(nmr[:], nmr[:], -1.0)
            rstd_bc = sb.tile([128, 1], F32, tag=f"rstd_bc_{name}")
            nc.gpsimd.partition_broadcast(rstd_bc[:], rstd[:])
            nmr_bc = sb.tile([128, 1], F32, tag=f"nmr_bc_{name}")
            nc.gpsimd.partition_broadcast(nmr_bc[:], nmr[:])
            return rstd_bc, nmr_bc

        def materialize_norm(src_T, dst_T, rstd_bc, nmr_bc):
            for dd in range(DT):
                nc.scalar.activation(dst_T[:, dd], src_T[:, dd], AF.Identity,
                                     bias=nmr_bc[:], scale=rstd_bc[:])

        # ================= Phase 0: staged loads =================
        # x + self-attn weights emitted inline (they gate phase 1);
        # later weights go on the background queue, pumped between heads.
        prep_act_steps(inp["data_dec"], x_T, engine="scalar")
        prep_w_steps("wq_m", wT["wq_m"])
        prep_w_steps("wk_m", wT["wk_m"])
        prep_w_steps("wv_m", wT["wv_m"])
        pump(len(bg))  # emit now: x, wq, wk, wv
        # ---- biases (f32, used as per-partition scalar operands) ----
        bias = {}
        for b in BNAMES + ["bf2"]:
            t = sb.tile([128, DT], F32, tag=f"{b}_sb")
            nc.sync.dma_start(t[:], inp[b].rearrange("(t p) -> p t", p=128))
            bias[b] = t
        bf1_sb = sb.tile([128, FT], F32, tag="bf1_sb")
        nc.sync.dma_start(bf1_sb[:], inp["bf1"].rearrange("(t p) -> p t", p=128))
        bv_full = {}
        for b in ("bv_m", "bv_c"):
            row = wstage()
            nc.sync.dma_start(row[0:1, :], inp[b][None, :])
            rowb = sb.tile([1, D], BF16, tag=f"{b}_rowb")
            nc.vector.tensor_copy(rowb[:], row[0:1, :])
            full = sb.tile([128, D], BF16, tag=f"{b}_full")
            nc.gpsimd.partition_broadcast(full[:], rowb[:])
            bv_full[b] = full

        prep_w_steps("wo_m", wT["wo_m"])
        pump(2)

        project_fm("wq_m", x_T, q_T, bias_tile=bias["bq_m"])
        project_fm("wk_m", x_T, k_T, bias_tile=bias["bk_m"],
                   out_engine="scalar")
        project_v("wv_m", "bv_m", x_T)
        pump(len(bg))  # rest of wo_m

        # warm up the collective stream so AllReduce #1 is not the first op
        ar_wi = dram.tile([1, 8], F32, tag="ar_wi")
        ar_wo = dram.tile([1, 8], F32, tag="ar_wo")
        warm8 = sb.tile([1, 8], F32, tag="warm8")
        nc.vector.memset(warm8[:], 0.0)
        nc.gpsimd.dma_start(ar_wi[:], warm8[:])
        nc.gpsimd.collective_compute(
            "AllReduce", OP.add, replica_groups=[list(range(N_CORES))],
            ins=[ar_wi.opt()], outs=[ar_wo.opt()])

        # queue cross-attn weights + enc for pumping inside self-attention
        prep_act_steps(inp["encoder_out"], enc_T)
        prep_w_steps("wk_c", wT["wk_c"])
        prep_w_steps("wv_c", wT["wv_c"])
        prep_w_steps("wq_c", wT["wq_c"], wsum=wsum_qc)
        prep_w_steps("wo_c", wT["wo_c"])

        # ================= Phase 1: self attention =================
        attention(q_T, k_T, attn_T, causal=True, pump_n=3)
        pump(len(bg))

        stats1 = sb.tile([128, 16], F32, tag="stats1")
        residual_out("wo_m", attn_T, bias["bo_m"], x_T, r1_T, stats1)
        g1 = stats_ar_kick(stats1, "n1")

        # ================= Phase 2: cross attention =================
        # k/v/q-raw projections (no AR dependency) overlap the AllReduce;
        # q's norm fix lands in-place afterwards so PSUM never backs up.
        project_fm("wk_c", enc_T, k_T, bias_tile=bias["bk_c"])
        project_v("wv_c", "bv_c", enc_T)
        project_fm("wq_c", r1_T, q_T, bias_tile=None, out_engine="scalar")
        rstd1, nmr1 = stats_ar_finish(g1, "n1")
        qfix = sb.tile([128, DT], F32, tag="qfix")
        for dd in range(DT):
            nc.vector.scalar_tensor_tensor(
                qfix[:, dd:dd + 1], wsum_qc[:, dd:dd + 1], nmr1[:],
                bias["bq_c"][:, dd:dd + 1], OP.mult, OP.add)
        for dd in range(DT):
            nc.vector.tensor_scalar(q_T[:, dd], q_T[:, dd], rstd1[:],
                                    qfix[:, dd:dd + 1], OP.mult, OP.add)

        # queue FFN weights; a few pumps fill the AR1/q-fix wait
        prep_wf1_steps()
        prep_wf2_steps()
        pump(6)

        attention(q_T, k_T, attn_T, causal=False, pump_n=3)

        nmm_T = sb.tile([128, DT, S], BF16, tag="g_x")     # reuses x_T space
        materialize_norm(r1_T, nmm_T, rstd1, nmr1)
        stats2 = sb.tile([128, 16], F32, tag="stats2")
        residual_out("wo_c", attn_T, bias["bo_c"], nmm_T, r2_T, stats2)
        g2 = stats_ar_kick(stats2, "n2")

        # ================= Phase 3: FFN =================
        # ff1 raw matmuls for BOTH halves run during the AllReduce; the
        # relu+scale pass (AR-dependent) is applied in-place on h_T.
        for th in range(TH):
            for ft in range(FT):
                pt = psB()
                for ki in range(DT):
                    nc.tensor.matmul(pt[:], wf1T[:, ki, ft * 128:(ft + 1) * 128],
                                     r2_T[:, ki, th * 512:(th + 1) * 512],
                                     start=(ki == 0), stop=(ki == DT - 1))
                nc.vector.tensor_copy(h_T[:, ft, th * 512:(th + 1) * 512],
                                      pt[:])
        pump(len(bg))  # remaining wf2 transposes fill the AllReduce #2 wait
        rstd2, nmr2 = stats_ar_finish(g2, "n2")
        ffix = sb.tile([128, FT], F32, tag="ffix")
        for ft in range(FT):
            nc.vector.scalar_tensor_tensor(
                ffix[:, ft:ft + 1], wsum_f1[:, ft:ft + 1], nmr2[:],
                bf1_sb[:, ft:ft + 1], OP.mult, OP.add)
        for ft in range(FT):
            if ft % 2 == 0:
                nc.scalar.activation(h_T[:, ft], h_T[:, ft], AF.Relu,
                                     bias=ffix[:, ft:ft + 1], scale=rstd2[:])
            else:
                nc.vector.tensor_scalar(h_T[:, ft], h_T[:, ft], rstd2[:],
                                        ffix[:, ft:ft + 1], OP.mult, OP.add)
                nc.vector.tensor_scalar_max(h_T[:, ft], h_T[:, ft], 0.0)
        nmh_T = sb.tile([128, DT, S], BF16, tag="g_enc")   # reuses enc_T space
        materialize_norm(r2_T, nmh_T, rstd2, nmr2)

        stats3 = sb.tile([128, 16], F32, tag="stats3")

        def r3_transpose(tt):
            pt = ps_b.tile([128, 512], BF16, tag="B", name="pT3")
            for dd in range(DT):
                nc.tensor.transpose(pt[:, dd * 128:(dd + 1) * 128],
                                    r3_T[:, dd, tt * 128:(tt + 1) * 128],
                                    ident_b[:])
            nc.vector.tensor_copy(
                r3_tok[:, tt].rearrange("p (k c) -> p k c", c=128),
                pt[:].rearrange("p (k c) -> p k c", c=128))

        for th in range(TH):
            for dd in range(DT):
                pt = psB()
                for ki in range(FT):
                    nc.tensor.matmul(pt[:], wf2T[:, ki, dd * 128:(dd + 1) * 128],
                                     h_T[:, ki, th * 512:(th + 1) * 512],
                                     start=(ki == 0), stop=(ki == FT - 1))
                dst = r3_T[:, dd, th * 512:(th + 1) * 512]
                c = dd * TH + th
                nc.vector.scalar_tensor_tensor(
                    dst, pt[:], bias["bf2"][:, dd:dd + 1],
                    nmh_T[:, dd, th * 512:(th + 1) * 512], OP.add, OP.add,
                    accum_out=stats3[:, c:c + 1])
                nc.vector.scalar_tensor_tensor(
                    scr[:], dst, 0.0, dst, OP.add, OP.mult,
                    accum_out=stats3[:, 8 + c:8 + c + 1])
        g3 = stats_ar_kick(stats3, "n3")
        # transpose r3 to token-major while AllReduce #3 is in flight
        for tt in range(TT):
            r3_transpose(tt)
        rstd3, nmr3 = stats_ar_finish(g3, "n3")
        for tt in range(TT):
            nc.vector.scalar_tensor_tensor(
                r3_tok[:, tt], r3_tok[:, tt], rstd3[:],
                nmr3[:, :].to_broadcast([128, D]), OP.mult, OP.add)
            nc.gpsimd.dma_start(
                out_d.rearrange("(tt p) d -> p tt d", p=128)[:, tt],
                r3_tok[:, tt])


_NC_CACHE = {}


def kernel(**inputs):
    if "nc" not in _NC_CACHE:
        _NC_CACHE["nc"] = build_nc()
    nc = _NC_CACHE["nc"]
    in_maps = []
    for b in range(N_CORES):
        m = {"data_dec": np.ascontiguousarray(
                 np.asarray(inputs["data_dec"], dtype=np.float32)[b]),
             "encoder_out": np.ascontiguousarray(
                 np.asarray(inputs["encoder_out"], dtype=np.float32)[b])}
        for k, v in inputs.items():
            if k not in ("data_dec", "encoder_out"):
                m[k] = np.ascontiguousarray(np.asarray(v, dtype=np.float32))
        in_maps.append(m)
    res = bass_utils.run_bass_kernel_spmd(nc, in_maps,
                                          core_ids=list(range(N_CORES)))
    return np.stack([res.results[b]["out"] for b in range(N_CORES)], axis=0)


# revision 16
# speedup vs baseline: 1.0895x; 1.0240x over previous
"""Trainium2 Bass kernel for nn_Decoder_Model (dense transformer decoder layer).

Sharding: data-parallel over batch (8 batches -> 8 cores). The three global
layernorms (normalized over ALL elements of the [B,S,D] tensor) need cross-core
scalar stats: each core computes local sum/sumsq, an 8-float AllReduce merges
them. AllReduce latency is hidden behind the next phase's matmuls using the
affine trick: norm(x)@W.T = (x@W.T)*rstd + per-channel-fix, so the big matmuls
run on raw x while stats are in flight and only a cheap fixup pass waits.

Perf notes vs the fp32r baseline:
- All matmul operands are bf16 (PSUM stays fp32): halves PE weight-load time,
  SBUF traffic and weight DMA; tolerance is 2e-2 so ~0.5% rounding is fine.
- Weights are transposed ONCE into resident SBUF (bf16), with psum->sbuf
  copy-outs grouped [128,512] and issued on gpsimd (scalar engine is reserved
  for softmax exp, its hard floor).
- Weight prep for later phases is pumped between attention heads so the PE
  never idles long enough to re-engage the HAM half-rate throttle.
- Cross-attn k/v projections are emitted before the q fix so AllReduce #1
  latency hides behind them.
"""
import sys

import numpy as np

sys.path.insert(0, "/opt/trn_rl_repo")

import concourse.bass as bass  # noqa: E402,F401
import concourse.mybir as mybir  # noqa: E402
import concourse.tile as tile  # noqa: E402
from concourse import bacc  # noqa: E402
from concourse import bass_utils  # noqa: E402
from concourse.masks import make_identity  # noqa: E402

F32 = mybir.dt.float32
F32R = mybir.dt.float32r
BF16 = mybir.dt.bfloat16
AF = mybir.ActivationFunctionType
OP = mybir.AluOpType

B, S, D, H, DK, FF = 8, 1024, 512, 8, 64, 2048
TT = S // 128   # 8 token tiles
DT = D // 128   # 4 feature tiles
FT = FF // 128  # 16 ffn tiles
TH = S // 512   # 2 matmul free-dim halves
N_CORES = 8
NTOT = float(B * S * D)
EPS = 1e-5

WNAMES = ["wq_m", "wk_m", "wv_m", "wo_m", "wq_c", "wk_c", "wv_c", "wo_c"]
BNAMES = ["bq_m", "bk_m", "bv_m", "bo_m", "bq_c", "bk_c", "bv_c", "bo_c"]

# self-attn causal chunking per k-tile kt over the q axis:
# (masked_chunk_start, masked_chunk_width, [(clean_start, clean_width), ...])
CAUSAL_CHUNKS = {
    0: (0, 256, [(256, 512), (768, 256)]),
    1: (128, 256, [(384, 384), (768, 256)]),
    2: (256, 256, [(512, 512)]),
    3: (384, 256, [(640, 384)]),
    4: (512, 256, [(768, 256)]),
    5: (640, 384, []),
    6: (768, 256, []),
    7: (896, 128, []),
}


def build_nc():
    nc = bacc.Bacc("TRN2", target_bir_lowering=False, debug=False,
                   enable_asserts=False, num_devices=N_CORES)
    inp = {}
    inp["data_dec"] = nc.dram_tensor("data_dec", [S, D], F32,
                                     kind="ExternalInput").ap()
    inp["encoder_out"] = nc.dram_tensor("encoder_out", [S, D], F32,
                                        kind="ExternalInput").ap()
    for w in WNAMES:
        inp[w] = nc.dram_tensor(w, [D, D], F32, kind="ExternalInput").ap()
    for b in BNAMES:
        inp[b] = nc.dram_tensor(b, [D], F32, kind="ExternalInput").ap()
    inp["wf1"] = nc.dram_tensor("wf1", [FF, D], F32, kind="ExternalInput").ap()
    inp["bf1"] = nc.dram_tensor("bf1", [FF], F32, kind="ExternalInput").ap()
    inp["wf2"] = nc.dram_tensor("wf2", [D, FF], F32, kind="ExternalInput").ap()
    inp["bf2"] = nc.dram_tensor("bf2", [D], F32, kind="ExternalInput").ap()
    out_d = nc.dram_tensor("out", [S, D], F32, kind="ExternalOutput").ap()

    with tile.TileContext(nc) as tc:
        build_body(nc, tc, inp, out_d)
    nc.finalize()
    return nc


def build_body(nc, tc, inp, out_d):
    import contextlib
    ctx = contextlib.ExitStack()
    with ctx:
        sb = ctx.enter_context(tc.tile_pool(name="sb", bufs=1))
        stg = ctx.enter_context(tc.tile_pool(name="stg", bufs=4))
        cp = ctx.enter_context(tc.tile_pool(name="cp", bufs=2))
        dram = ctx.enter_context(tc.tile_pool(name="dram", bufs=1, space="DRAM"))
        ps_a = ctx.enter_context(tc.tile_pool(name="ps_a", bufs=3, space="PSUM"))
        ps_b = ctx.enter_context(tc.tile_pool(name="ps_b", bufs=3, space="PSUM"))
        ps_pv = ctx.enter_context(tc.tile_pool(name="ps_pv", bufs=2, space="PSUM"))

        def psA():
            return ps_a.tile([128, 512], F32, tag="A", name="pA")

        def psB():
            return ps_b.tile([128, 512], F32, tag="B", name="pB")

        def psT():
            return ps_b.tile([128, 512], F32, tag="B", name="pT")

        def wstage():
            return stg.tile([128, 512], F32, tag="wstage", name="wstg")

        ident = sb.tile([128, 128], F32, tag="ident")
        make_identity(nc, ident[:])
        ident_b = sb.tile([128, 128], BF16, tag="ident_b")
        nc.vector.tensor_copy(ident_b[:], ident[:])
        onesf = sb.tile([128, 1], F32, tag="onesf")
        nc.vector.memset(onesf[:], 1.0)

        # column sums for the norm affine fixes
        wsum_qc = sb.tile([128, DT], F32, tag="wsum_qc")
        wsum_f1 = sb.tile([128, FT], F32, tag="wsum_f1")

        # ---- resident transposed weights (bf16) ----
        wT = {w: sb.tile([128, DT, D], BF16, tag=f"T_{w}", name=f"T_{w}")
              for w in WNAMES}
        wf1T = sb.tile([128, DT, FF], BF16, tag="T_wf1")
        wf2T = sb.tile([128, FT, D], BF16, tag="T_wf2")

        # ---- activations ----
        x_T = sb.tile([128, DT, S], BF16, tag="g_x")
        enc_T = sb.tile([128, DT, S], BF16, tag="g_enc")
        q_T = sb.tile([128, DT, S], BF16, tag="g_q")
        k_T = sb.tile([128, DT, S], BF16, tag="g_k")
        v_tok = sb.tile([128, TT, H * 65], BF16, tag="g_v")
        attn_T = sb.tile([128, DT, S], BF16, tag="g_attn")
        r1_T = sb.tile([128, DT, S], BF16, tag="g_r1")
        r2_T = sb.tile([128, DT, S], BF16, tag="g_r2")
        h_T = sb.tile([128, FT, S], BF16, tag="g_h")
        r3_T = sb.tile([128, DT, S], BF16, tag="g_r3")
        r3_tok = sb.tile([128, TT, D], BF16, tag="g_r3tok")
        scr = sb.tile([128, 512], F32, tag="scr")

        # ---- transpose helpers ----
        def transpose_group4(dst_view, stage, wsum_col=None, engine="vector"):
            """stage [128(rows),512(=4x128 cols)] -> 4 transposed blocks into
            one psum bank, one grouped copy-out to dst_view
            ([128, 4, 128] view of a bf16 resident tile)."""
            pt = psT()
            for ki in range(4):
                nc.tensor.transpose(pt[:, ki * 128:(ki + 1) * 128],
                                    stage[:, ki * 128:(ki + 1) * 128], ident[:])
            pv_ = pt[:].rearrange("p (k c) -> p k c", c=128)
            if engine == "scalar":
                nc.scalar.copy(dst_view, pv_)
            else:
                nc.vector.tensor_copy(dst_view, pv_)
            if wsum_col is not None:
                nc.vector.reduce_sum(wsum_col, stage[:],
                                     axis=mybir.AxisListType.X)

        def stage_dma(src_ap):
            stage = wstage()
            nc.sync.dma_start(stage[:], src_ap)
            return stage

        # background work queue: each closure emits one stage of weight prep
        bg = []

        def pump(n):
            for _ in range(min(n, len(bg))):
                bg.pop(0)()

        def prep_w_steps(wname, dst, wsum=None):
            """[512,512] weight -> dst [128, DT, 512] transposed bf16."""
            for ot in range(DT):
                def step(ot=ot):
                    stage = stage_dma(
                        inp[wname].rearrange("(t p) i -> p t i", p=128)[:, ot])
                    wcol = wsum[:, ot:ot + 1] if wsum is not None else None
                    transpose_group4(dst[:, :, ot * 128:(ot + 1) * 128],
                                     stage, wcol)
                bg.append(step)

        def prep_wf1_steps():
            for ot in range(FT):
                def step(ot=ot):
                    stage = stage_dma(
                        inp["wf1"].rearrange("(t p) i -> p t i", p=128)[:, ot])
                    transpose_group4(wf1T[:, :, ot * 128:(ot + 1) * 128],
                                     stage, wsum_f1[:, ot:ot + 1])
                bg.append(step)

        def prep_wf2_steps():
            for dd in range(DT):
                for piece in range(4):
                    def step(dd=dd, piece=piece):
                        stage = stage_dma(
                            inp["wf2"].rearrange("(t p) i -> p t i", p=128)
                            [:, dd, piece * 512:(piece + 1) * 512])
                        transpose_group4(
                            wf2T[:, piece * 4:piece * 4 + 4,
                                 dd * 128:(dd + 1) * 128], stage)
                    bg.append(step)

        def prep_act_steps(src_d, dst_T, engine="vector"):
            """[S,D] activation -> dst_T [128, DT, S] bf16 feature-major."""
            for tt in range(TT):
                def step(tt=tt):
                    stage = stage_dma(
                        src_d.rearrange("(tt p) d -> p tt d", p=128)[:, tt])
                    transpose_group4(dst_T[:, :, tt * 128:(tt + 1) * 128],
                                     stage, engine=engine)
                bg.append(step)

        # ---- projection helpers ----
        def project_fm(w, src_T, out_tile, bias_tile=None, out_engine="vector"):
            """Feature-major projection: out[:, dd, :] = W^T-block @ src.
            bias_tile=None leaves the output raw (bias/norm fix applied later
            in-place, so an AllReduce wait never backs up the PSUM banks)."""
            for dd in range(DT):
                for th in range(TH):
                    pt = psB()
                    for ki in range(DT):
                        nc.tensor.matmul(pt[:], wT[w][:, ki, dd * 128:(dd + 1) * 128],
                                         src_T[:, ki, th * 512:(th + 1) * 512],
                                         start=(ki == 0), stop=(ki == DT - 1))
                    dst = out_tile[:, dd, th * 512:(th + 1) * 512]
                    if out_engine == "scalar":
                        if bias_tile is None:
                            nc.scalar.copy(dst, pt[:])
                        else:
                            nc.scalar.activation(dst, pt[:], AF.Identity,
                                                 bias=bias_tile[:, dd:dd + 1])
                    else:
                        if bias_tile is None:
                            nc.vector.tensor_copy(dst, pt[:])
                        else:
                            nc.vector.tensor_scalar(dst, pt[:],
                                                    bias_tile[:, dd:dd + 1],
                                                    None, OP.add)

        def project_v(w, bname, src_T):
            """Token-major v with per-head ones column: v_tok [128,TT,H*65]."""
            ones_view = v_tok[:, :, :].rearrange(
                "p t (h c) -> p t h c", c=65)[:, :, :, 64]
            nc.vector.tensor_copy(
                ones_view, onesf[:, 0:1, None].to_broadcast([128, TT, H]))
            for tt in range(TT):
                pt = psB()
                for ki in range(DT):
                    nc.tensor.matmul(pt[:], src_T[:, ki, tt * 128:(tt + 1) * 128],
                                     wT[w][:, ki],
                                     start=(ki == 0), stop=(ki == DT - 1))
                dstv = v_tok[:, tt].rearrange("p (h c) -> p h c", c=65)[:, :, 0:64]
                nc.vector.tensor_tensor(
                    dstv, pt[:].rearrange("p (h c) -> p h c", c=64),
                    bv_full[bname][:].rearrange("p (h c) -> p h c", c=64),
                    OP.add)

        def attention(q_t, k_t, attn_t, causal, pump_n=0):
            for h in range(H):
                dt_, base = h // 2, (h % 2) * 64
                q_h = q_t[base:base + 64, dt_]
                k_h = k_t[base:base + 64, dt_]
                pv = {qh: ps_pv.tile([128, 512], F32, tag="PV", name="pPV")
                      for qh in range(TH)}
                for kt in range(TT):
                    pr = cp.tile([128, S], BF16, tag="probs", name="probs")
                    if causal:
                        m0, mw, clean = CAUSAL_CHUNKS[kt]
                        chunks = [(m0, mw, True)] + [(c0, cw, False)
                                                     for (c0, cw) in clean]
                    else:
                        chunks = [(0, 512, False), (512, 512, False)]
                    for (c0, cw, masked) in chunks:
                        st = psA()
                        nc.tensor.matmul(st[:, :cw],
                                         k_h[:, kt * 128:(kt + 1) * 128],
                                         q_h[:, c0:c0 + cw],
                                         start=True, stop=True)
                        nc.scalar.activation(pr[:, c0:c0 + cw], st[:, :cw],
                                             AF.Exp, scale=1.0 / 32.0)
                        if masked:
                            # only the first 128 cols are the triangle block
                            nc.gpsimd.affine_select(
                                out=pr[:, c0:c0 + 128], in_=pr[:, c0:c0 + 128],
                                compare_op=OP.is_ge, fill=0.0, base=0,
                                channel_multiplier=-1, pattern=[[1, 128]])
                    # PV contributions of this kt
                    v_h = v_tok[:, kt, h * 65:(h + 1) * 65]
                    for qh in range(TH):
                        if causal and qh == 0 and kt > 3:
                            continue
                        if causal:
                            off = max(0, (kt - qh * 4) * 128)
                            last = (kt == 3) if qh == 0 else (kt == 7)
                        else:
                            off, last = 0, (kt == 7)
                        nc.tensor.matmul(
                            pv[qh][:65, off:512], v_h,
                            pr[:, qh * 512 + off:(qh + 1) * 512],
                            start=(kt == 0), stop=last)
                # copy pv out of PSUM at once (frees the bank for the next
                # head), then normalize by the rowsum in row 64 from SBUF
                for qh in range(TH):
                    pvs = cp.tile([65, 512], F32, tag="pvstage", name="pvs")
                    nc.vector.tensor_copy(pvs[:], pv[qh][:65, :])
                    rec = cp.tile([1, 512], F32, tag="rsrec", name="rec", bufs=1)
                    nc.vector.reciprocal(rec[:], pvs[64:65, :])
                    rb = cp.tile([64, 512], F32, tag="rsbc", name="rb")
                    nc.gpsimd.partition_broadcast(rb[:], rec[:])
                    nc.vector.tensor_tensor(
                        attn_t[base:base + 64, dt_, qh * 512:(qh + 1) * 512],
                        pvs[0:64, :], rb[:], OP.mult)
                if pump_n:
                    pump(pump_n)

        def residual_out(w, src_T, bias_tile, res_T, out_T, stats_sb):
            """out_T = (W^T @ src_T) + bias + res_T ; accumulate sum/sumsq."""
            n_ki = src_T.shape[1]
            for dd in range(DT):
                for th in range(TH):
                    pt = psB()
                    for ki in range(n_ki):
                        nc.tensor.matmul(pt[:], wT[w][:, ki, dd * 128:(dd + 1) * 128],
                                         src_T[:, ki, th * 512:(th + 1) * 512],
                                         start=(ki == 0), stop=(ki == n_ki - 1))
                    dst = out_T[:, dd, th * 512:(th + 1) * 512]
                    c = dd * TH + th
                    nc.vector.scalar_tensor_tensor(
                        dst, pt[:], bias_tile[:, dd:dd + 1],
                        res_T[:, dd, th * 512:(th + 1) * 512],
                        OP.add, OP.add, accum_out=stats_sb[:, c:c + 1])
                    nc.vector.scalar_tensor_tensor(
                        scr[:], dst, 0.0, dst, OP.add, OP.mult,
                        accum_out=stats_sb[:, 8 + c:8 + c + 1])

        def stats_ar_kick(stats_sb, name):
            pt = psA()
            nc.tensor.matmul(pt[:1, :16], onesf[:], stats_sb[:],
                             start=True, stop=True)
            red = sb.tile([1, 8], F32, tag=f"red_{name}")
            nc.vector.reduce_sum(red[:, 0:1], pt[0:1, 0:8],
                                 axis=mybir.AxisListType.X)
            nc.vector.reduce_sum(red[:, 1:2], pt[0:1, 8:16],
                                 axis=mybir.AxisListType.X)
            nc.vector.memset(red[:, 2:8], 0.0)
            ar_in = dram.tile([1, 8], F32, tag=f"ar_in_{name}")
            ar_out = dram.tile([1, 8], F32, tag=f"ar_out_{name}")
            nc.gpsimd.dma_start(ar_in[:], red[:])
            nc.gpsimd.collective_compute(
                "AllReduce", OP.add, replica_groups=[list(range(N_CORES))],
                ins=[ar_in.opt()], outs=[ar_out.opt()])
            g = sb.tile([1, 8], F32, tag=f"g_{name}")
            nc.sync.dma_start(g[:], ar_out[:])
            return g

        def stats_ar_finish(g, name):
            mu = sb.tile([1, 1], F32, tag=f"mu_{name}")
            nc.vector.tensor_scalar_mul(mu[:], g[:, 0:1], 1.0 / NTOT)
            ex2 = sb.tile([1, 1], F32, tag=f"ex2_{name}")
            nc.vector.tensor_scalar_mul(ex2[:], g[:, 1:2], 1.0 / NTOT)
            mu2 = sb.tile([1, 1], F32, tag=f"mu2_{name}")
            nc.vector.tensor_tensor(mu2[:], mu[:], mu[:], OP.mult)
            var = sb.tile([1, 1], F32, tag=f"var_{name}")
            nc.vector.tensor_tensor(var[:], ex2[:], mu2[:], OP.subtract)
            epst = sb.tile([1, 1], F32, tag=f"eps_{name}")
            nc.vector.memset(epst[:], EPS)
            std = sb.tile([1, 1], F32, tag=f"std_{name}")
            nc.scalar.activation(std[:], var[:], AF.Sqrt, bias=epst[:])
            rstd = sb.tile([1, 1], F32, tag=f"rstd_{name}")
            nc.vector.reciprocal(rstd[:], std[:])
            nmr = sb.tile([1, 1], F32, tag=f"nmr_{name}")
            nc.vector.tensor_tensor(nmr[:], mu[:], rstd[:], OP.mult)
            nc.vector.tensor_scalar_mul(nmr[:], nmr[:], -1.0)
            rstd_bc = sb.tile([128, 1], F32, tag=f"rstd_bc_{name}")
            nc.gpsimd.partition_broadcast(rstd_bc[:], rstd[:])
            nmr_bc = sb.tile([128, 1], F32, tag=f"nmr_bc_{name}")
            nc.gpsimd.partition_broadcast(nmr_bc[:], nmr[:])
            return rstd_bc, nmr_bc

        def materialize_norm(src_T, dst_T, rstd_bc, nmr_bc):
            for dd in range(DT):
                nc.scalar.activation(dst_T[:, dd], src_T[:, dd], AF.Identity,
                                     bias=nmr_bc[:], scale=rstd_bc[:])

        # ================= Phase 0: staged loads =================
        # x + self-attn weights emitted inline (they gate phase 1);
        # later weights go on the background queue, pumped between heads.
        prep_act_steps(inp["data_dec"], x_T, engine="scalar")
        prep_w_steps("wq_m", wT["wq_m"])
        prep_w_steps("wk_m", wT["wk_m"])
        prep_w_steps("wv_m", wT["wv_m"])
        pump(len(bg))  # emit now: x, wq, wk, wv
        # ---- biases (f32, used as per-partition scalar operands) ----
        bias = {}
        for b in BNAMES + ["bf2"]:
            t = sb.tile([128, DT], F32, tag=f"{b}_sb")
            nc.gpsimd.dma_start(t[:], inp[b].rearrange("(t p) -> p t", p=128))
            bias[b] = t
        bf1_sb = sb.tile([128, FT], F32, tag="bf1_sb")
        nc.gpsimd.dma_start(bf1_sb[:], inp["bf1"].rearrange("(t p) -> p t", p=128))
        bv_full = {}
        for b in ("bv_m", "bv_c"):
            row = wstage()
            nc.gpsimd.dma_start(row[0:1, :], inp[b][None, :])
            rowb = sb.tile([1, D], BF16, tag=f"{b}_rowb")
            nc.vector.tensor_copy(rowb[:], row[0:1, :])
            full = sb.tile([128, D], BF16, tag=f"{b}_full")
            nc.gpsimd.partition_broadcast(full[:], rowb[:])
            bv_full[b] = full

        prep_w_steps("wo_m", wT["wo_m"])
        pump(2)

        project_fm("wq_m", x_T, q_T, bias_tile=bias["bq_m"])
        project_fm("wk_m", x_T, k_T, bias_tile=bias["bk_m"],
                   out_engine="scalar")
        project_v("wv_m", "bv_m", x_T)
        pump(len(bg))  # rest of wo_m

        # warm up the collective stream so AllReduce #1 is not the first op
        ar_wi = dram.tile([1, 8], F32, tag="ar_wi")
        ar_wo = dram.tile([1, 8], F32, tag="ar_wo")
        warm8 = sb.tile([1, 8], F32, tag="warm8")
        nc.vector.memset(warm8[:], 0.0)
        nc.gpsimd.dma_start(ar_wi[:], warm8[:])
        nc.gpsimd.collective_compute(
            "AllReduce", OP.add, replica_groups=[list(range(N_CORES))],
            ins=[ar_wi.opt()], outs=[ar_wo.opt()])

        # queue cross-attn weights + enc for pumping inside self-attention
        prep_act_steps(inp["encoder_out"], enc_T)
        prep_w_steps("wk_c", wT["wk_c"])
        prep_w_steps("wv_c", wT["wv_c"])
        prep_w_steps("wq_c", wT["wq_c"], wsum=wsum_qc)
        prep_w_steps("wo_c", wT["wo_c"])

        # ================= Phase 1: self attention =================
        attention(q_T, k_T, attn_T, causal=True, pump_n=3)
        pump(len(bg))

        stats1 = sb.tile([128, 16], F32, tag="stats1")
        residual_out("wo_m", attn_T, bias["bo_m"], x_T, r1_T, stats1)
        g1 = stats_ar_kick(stats1, "n1")

        # ================= Phase 2: cross attention =================
        # k/v/q-raw projections (no AR dependency) overlap the AllReduce;
        # q's norm fix lands in-place afterwards so PSUM never backs up.
        project_fm("wk_c", enc_T, k_T, bias_tile=bias["bk_c"])
        project_v("wv_c", "bv_c", enc_T)
        project_fm("wq_c", r1_T, q_T, bias_tile=None, out_engine="scalar")
        rstd1, nmr1 = stats_ar_finish(g1, "n1")
        qfix = sb.tile([128, DT], F32, tag="qfix")
        for dd in range(DT):
            nc.vector.scalar_tensor_tensor(
                qfix[:, dd:dd + 1], wsum_qc[:, dd:dd + 1], nmr1[:],
                bias["bq_c"][:, dd:dd + 1], OP.mult, OP.add)
        for dd in range(DT):
            nc.vector.tensor_scalar(q_T[:, dd], q_T[:, dd], rstd1[:],
                                    qfix[:, dd:dd + 1], OP.mult, OP.add)

        # queue FFN weights; a few pumps fill the AR1/q-fix wait
        prep_wf1_steps()
        prep_wf2_steps()
        pump(6)

        attention(q_T, k_T, attn_T, causal=False, pump_n=3)

        nmm_T = sb.tile([128, DT, S], BF16, tag="g_x")     # reuses x_T space
        materialize_norm(r1_T, nmm_T, rstd1, nmr1)
        stats2 = sb.tile([128, 16], F32, tag="stats2")
        residual_out("wo_c", attn_T, bias["bo_c"], nmm_T, r2_T, stats2)
        g2 = stats_ar_kick(stats2, "n2")

        # ================= Phase 3: FFN =================
        # ff1 raw matmuls for BOTH halves run during the AllReduce; the
        # relu+scale pass (AR-dependent) is applied in-place on h_T.
        for th in range(TH):
            for ft in range(FT):
                pt = psB()
                for ki in range(DT):
                    nc.tensor.matmul(pt[:], wf1T[:, ki, ft * 128:(ft + 1) * 128],
                                     r2_T[:, ki, th * 512:(th + 1) * 512],
                                     start=(ki == 0), stop=(ki == DT - 1))
                nc.vector.tensor_copy(h_T[:, ft, th * 512:(th + 1) * 512],
                                      pt[:])
        pump(len(bg))  # remaining wf2 transposes fill the AllReduce #2 wait
        rstd2, nmr2 = stats_ar_finish(g2, "n2")
        ffix = sb.tile([128, FT], F32, tag="ffix")
        for ft in range(FT):
            nc.vector.scalar_tensor_tensor(
                ffix[:, ft:ft + 1], wsum_f1[:, ft:ft + 1], nmr2[:],
                bf1_sb[:, ft:ft + 1], OP.mult, OP.add)
        for ft in range(FT):
            if ft % 2 == 0:
                nc.scalar.activation(h_T[:, ft], h_T[:, ft], AF.Relu,
                                     bias=ffix[:, ft:ft + 1], scale=rstd2[:])
            else:
                nc.vector.tensor_scalar(h_T[:, ft], h_T[:, ft], rstd2[:],
                                        ffix[:, ft:ft + 1], OP.mult, OP.add)
                nc.vector.tensor_scalar_max(h_T[:, ft], h_T[:, ft], 0.0)
        nmh_T = sb.tile([128, DT, S], BF16, tag="g_enc")   # reuses enc_T space
        materialize_norm(r2_T, nmh_T, rstd2, nmr2)

        stats3 = sb.tile([128, 16], F32, tag="stats3")

        def r3_transpose(tt):
            pt = ps_b.tile([128, 512], BF16, tag="B", name="pT3")
            for dd in range(DT):
                nc.tensor.transpose(pt[:, dd * 128:(dd + 1) * 128],
                                    r3_T[:, dd, tt * 128:(tt + 1) * 128],
                                    ident_b[:])
            nc.vector.tensor_copy(
                r3_tok[:, tt].rearrange("p (k c) -> p k c", c=128),
                pt[:].rearrange("p (k c) -> p k c", c=128))

        for th in range(TH):
            for dd in range(DT):
                pt = psB()
                for ki in range(FT):
                    nc.tensor.matmul(pt[:], wf2T[:, ki, dd * 128:(dd + 1) * 128],
                                     h_T[:, ki, th * 512:(th + 1) * 512],
                                     start=(ki == 0), stop=(ki == FT - 1))
                dst = r3_T[:, dd, th * 512:(th + 1) * 512]
                c = dd * TH + th
                nc.vector.scalar_tensor_tensor(
                    dst, pt[:], bias["bf2"][:, dd:dd + 1],
                    nmh_T[:, dd, th * 512:(th + 1) * 512], OP.add, OP.add,
                    accum_out=stats3[:, c:c + 1])
                nc.vector.scalar_tensor_tensor(
                    scr[:], dst, 0.0, dst, OP.add, OP.mult,
                    accum_out=stats3[:, 8 + c:8 + c + 1])
        g3 = stats_ar_kick(stats3, "n3")
        # transpose r3 to token-major while AllReduce #3 is in flight
        for tt in range(TT):
            r3_transpose(tt)
        rstd3, nmr3 = stats_ar_finish(g3, "n3")
        for tt in range(TT):
            ost = wstage()
            if tt % 2 == 0:
                nc.scalar.activation(ost[:], r3_tok[:, tt], AF.Identity,
                                     bias=nmr3[:], scale=rstd3[:])
            else:
                nc.vector.scalar_tensor_tensor(
                    ost[:], r3_tok[:, tt], rstd3[:],
                    nmr3[:, :].to_broadcast([128, D]), OP.mult, OP.add)
            nc.sync.dma_start(
                out_d.rearrange("(tt p) d -> p tt d", p=128)[:, tt],
                ost[:])


_NC_CACHE = {}


def kernel(**inputs):
    if "nc" not in _NC_CACHE:
        _NC_CACHE["nc"] = build_nc()
    nc = _NC_CACHE["nc"]
    in_maps = []
    for b in range(N_CORES):
        m = {"data_dec": np.ascontiguousarray(
                 np.asarray(inputs["data_dec"], dtype=np.float32)[b]),
             "encoder_out": np.ascontiguousarray(
                 np.asarray(inputs["encoder_out"], dtype=np.float32)[b])}
        for k, v in inputs.items():
            if k not in ("data_dec", "encoder_out"):
                m[k] = np.ascontiguousarray(np.asarray(v, dtype=np.float32))
        in_maps.append(m)
    res = bass_utils.run_bass_kernel_spmd(nc, in_maps,
                                          core_ids=list(range(N_CORES)))
    return np.stack([res.results[b]["out"] for b in range(N_CORES)], axis=0)
